# revision 1
# baseline (speedup 1.0000x reference)
"""CrossConsensus kernel for 8 Trainium2 NeuronCores.

Sharding: data-parallel over B*L rows. Core c handles batch b=c//4,
target rows [ (c%4)*512, (c%4+1)*512 ).  All computation is row-local
(edge_i = repeat(arange(L), 8) means each edge scatters back to its own
source row), so there are no collectives; each core needs its target
row-chunk plus the full context of its batch.
"""

import math

import numpy as np

import concourse.bass as bass
import concourse.bacc as bacc
import concourse.tile as tile
from concourse import mybir
from concourse.bass_utils import run_bass_kernel_spmd
from concourse.masks import make_identity

F32 = mybir.dt.float32
BF16 = mybir.dt.bfloat16
U32 = mybir.dt.uint32
AX = mybir.AxisListType
ALU = mybir.AluOpType
ACTF = mybir.ActivationFunctionType

# problem constants (hardcoded per the harness contract)
B, L, K, D = 2, 2048, 2048, 512
H, R, WWIN, T, EH = 8, 8, 8, 2, 16
HD = D // H            # 64
LC = L * B // 8        # 512 rows per core
NT = LC // 128         # 4 l-tiles per core
KT = K // 128          # 16 k-tiles
CROW = D + 2 * EH      # 544: gather-table row [v(512) | ca(16) | cl(16)]
R9 = R + 1             # rank slots incl. alpha slot
TWO_PI = 2.0 * math.pi


def build_program():
    nc = bacc.Bacc()

    # ---------------- external I/O ----------------
    tT = nc.dram_tensor("tT", [D, LC], F32, kind="ExternalInput")        # target^T
    cT = nc.dram_tensor("cT", [D, K], F32, kind="ExternalInput")         # context^T
    Wt_d = nc.dram_tensor("Wt", [D, D], F32, kind="ExternalInput")
    WtR_d = nc.dram_tensor("WtR", [D, D], F32, kind="ExternalInput")     # rotate_half-folded
    Wc_d = nc.dram_tensor("Wc", [D, D], F32, kind="ExternalInput")
    Wo_d = nc.dram_tensor("Wo", [D, D], F32, kind="ExternalInput")
    bt_d = nc.dram_tensor("bt", [1, D], F32, kind="ExternalInput")
    btR_d = nc.dram_tensor("btR", [1, D], F32, kind="ExternalInput")
    bc_d = nc.dram_tensor("bc", [1, D], F32, kind="ExternalInput")
    bo_d = nc.dram_tensor("bo", [1, D], F32, kind="ExternalInput")
    Wtr3_d = nc.dram_tensor("Wtr3", [D, 48], F32, kind="ExternalInput")  # [Ws1t|Wa1t|Wl1t]
    Ws1c_d = nc.dram_tensor("Ws1c", [D, EH], F32, kind="ExternalInput")
    Wacl_d = nc.dram_tensor("Wacl", [D, 32], F32, kind="ExternalInput")  # [Wa1c|Wl1c]
    bs1_d = nc.dram_tensor("bs1", [1, EH], F32, kind="ExternalInput")
    bacl_d = nc.dram_tensor("bacl", [1, 32], F32, kind="ExternalInput")  # [ba1|bl1]
    Ws2bd_d = nc.dram_tensor("Ws2bd", [128, 8], F32, kind="ExternalInput")
    Wa2_d = nc.dram_tensor("Wa2", [1, EH], F32, kind="ExternalInput")
    ba2_d = nc.dram_tensor("ba2", [1, 1], F32, kind="ExternalInput")
    Wl2_d = nc.dram_tensor("Wl2", [128, H * R * HD], BF16, kind="ExternalInput")
    stp_d = nc.dram_tensor("stp", [128, T * NT], F32, kind="ExternalInput")
    lcol_d = nc.dram_tensor("lcol", [128, NT], F32, kind="ExternalInput")
    invf_d = nc.dram_tensor("invf", [1, HD // 2], F32, kind="ExternalInput")
    y_d = nc.dram_tensor("y", [LC, D], F32, kind="ExternalOutput")

    # internal DRAM gather table
    Tctx = nc.dram_tensor("Tctx", [K, CROW], BF16)

    # ---------------- persistent SBUF (static allocs, before pools) ----------
    ident = nc.alloc_sbuf_tensor("ident", [128, 128], F32).ap()
    ones1 = nc.alloc_sbuf_tensor("ones1", [1, 512], F32).ap()
    u_sb = [nc.alloc_sbuf_tensor(f"u{i}", [128, D], F32).ap() for i in range(NT)]
    uR_sb = [nc.alloc_sbuf_tensor(f"uR{i}", [128, D], F32).ap() for i in range(NT)]
    trio = [nc.alloc_sbuf_tensor(f"trio{i}", [128, 48], F32).ap() for i in range(NT)]
    Wl2_sb = nc.alloc_sbuf_tensor("Wl2sb", [128, H * R * HD], BF16).ap()
    cpTrep = nc.alloc_sbuf_tensor("cpTrep", [128, K], F32).ap()
    tpbT = nc.alloc_sbuf_tensor("tpbT", [128, NT * 16], F32).ap()
    invf_sb = nc.alloc_sbuf_tensor("invfsb", [128, HD // 2], F32).ap()
    wa2_sb = nc.alloc_sbuf_tensor("wa2sb", [128, EH], F32).ap()
    ba2_sb = nc.alloc_sbuf_tensor("ba2sb", [128, 1], F32).ap()
    stp_sb = nc.alloc_sbuf_tensor("stpsb", [128, T * NT], F32).ap()
    stpn_sb = nc.alloc_sbuf_tensor("stpnsb", [128, T * NT], F32).ap()
    lcol_sb = nc.alloc_sbuf_tensor("lcolsb", [128, NT], F32).ap()
    bs1_sb = nc.alloc_sbuf_tensor("bs1sb", [1, EH], F32).ap()
    bacl_sb = nc.alloc_sbuf_tensor("baclsb", [1, 32], F32).ap()
    bt_sb = nc.alloc_sbuf_tensor("btsb", [1, D], F32).ap()
    btR_sb = nc.alloc_sbuf_tensor("btRsb", [1, D], F32).ap()
    bc_sb = nc.alloc_sbuf_tensor("bcsb", [1, D], F32).ap()
    bo_sb = nc.alloc_sbuf_tensor("bosb", [1, D], F32).ap()
    Ws2bd_sb = nc.alloc_sbuf_tensor("ws2bdsb", [128, 8], F32).ap()
    Wtr3_sb = nc.alloc_sbuf_tensor("wtr3sb", [128, 4 * 48], F32).ap()
    Wacl_sb = nc.alloc_sbuf_tensor("waclsb", [128, 4 * 32], F32).ap()
    halfpi = nc.alloc_sbuf_tensor("halfpi", [128, 1], F32).ap()
    onec = nc.alloc_sbuf_tensor("onec", [128, 1], F32).ap()
    onesb = nc.alloc_sbuf_tensor("onesb", [1, 256], BF16).ap()

    with tile.TileContext(nc) as tc:
        with (
            tc.tile_pool(name="ld", bufs=3) as ldp,             # small staging tiles
            tc.tile_pool(name="gbp", bufs=1) as gbp,            # gather block
            tc.tile_pool(name="lamp", bufs=2) as lamp,          # Lam
            tc.tile_pool(name="prodp", bufs=2) as prodp,        # einsum products
            tc.tile_pool(name="med", bufs=2) as medp,
            tc.tile_pool(name="sml", bufs=2) as smlp,
            tc.tile_pool(name="wp", bufs=1) as wp,
            tc.tile_pool(name="ps", bufs=2, space="PSUM") as psp,
            tc.tile_pool(name="ps4", bufs=4, space="PSUM") as ps4p,
        ):
            # ---------- constants ----------
            make_identity(nc, ident)
            nc.vector.memset(ones1, 1.0)
            nc.vector.memset(halfpi, math.pi / 2)
            nc.vector.memset(onec, 1.0)
            nc.vector.memset(onesb, 1.0)
            nc.sync.dma_start(out=invf_sb, in_=invf_d[:].partition_broadcast(128))
            nc.sync.dma_start(out=wa2_sb, in_=Wa2_d[:].partition_broadcast(128))
            nc.sync.dma_start(out=ba2_sb, in_=ba2_d[:].partition_broadcast(128))
            nc.sync.dma_start(out=lcol_sb, in_=lcol_d[:])
            nc.sync.dma_start(out=bs1_sb, in_=bs1_d[:])
            nc.sync.dma_start(out=bacl_sb, in_=bacl_d[:])
            nc.sync.dma_start(out=bt_sb, in_=bt_d[:])
            nc.sync.dma_start(out=btR_sb, in_=btR_d[:])
            nc.sync.dma_start(out=bc_sb, in_=bc_d[:])
            nc.sync.dma_start(out=bo_sb, in_=bo_d[:])
            nc.sync.dma_start(out=Ws2bd_sb, in_=Ws2bd_d[:])
            nc.sync.dma_start(out=Wl2_sb, in_=Wl2_d[:])
            for dc in range(4):
                sl = slice(dc * 128, (dc + 1) * 128)
                nc.sync.dma_start(out=Wtr3_sb[:, dc * 48:(dc + 1) * 48], in_=Wtr3_d[sl, :])
                nc.sync.dma_start(out=Wacl_sb[:, dc * 32:(dc + 1) * 32], in_=Wacl_d[sl, :])

            def load_w(dram):
                t = wp.tile([128, 4 * D], F32, tag="wrhs")
                for dc in range(4):
                    nc.sync.dma_start(out=t[:, dc * D:(dc + 1) * D],
                                      in_=dram[dc * 128:(dc + 1) * 128, :])
                return t

            def softplus(dst, src, bias_ap, tmp_pool, tmp_tag):
                """dst = softplus(src + bias) = relu(x) + ln(1+exp(-|x|)).
                No softplus HW table; composed from abs/exp/ln (one table set)."""
                shp = [src.shape[0], src.free_size()]
                a = tmp_pool.tile(shp, F32, tag=tmp_tag)
                if bias_ap is None:
                    nc.scalar.activation(a[:], src, ACTF.Abs)
                    nc.vector.tensor_scalar(dst, src, 0.0, scalar2=None, op0=ALU.max)
                else:
                    nc.scalar.activation(a[:], src, ACTF.Abs, bias=bias_ap)
                    nc.vector.tensor_scalar(dst, src, bias_ap, scalar2=0.0,
                                            op0=ALU.add, op1=ALU.max)
                nc.scalar.activation(a[:], a[:], ACTF.Exp, scale=-1.0)
                nc.scalar.activation(a[:], a[:], ACTF.Ln, bias=onec[:, 0:1])
                nc.vector.tensor_tensor(dst, dst, a[:], op=ALU.add)

            stp_raw = smlp.tile([128, T * NT], F32, tag="stpraw")
            nc.sync.dma_start(out=stp_raw[:], in_=stp_d[:])
            softplus(stp_sb, stp_raw[:], None, smlp, "sptmp")
            nc.vector.tensor_scalar_mul(stpn_sb, stp_sb, -1.0)

            # ---------- dense projections ----------
            def mm_rows(out_ap, lhsT_dram, tix, w_sb, ncol, bias_sb, evac="v"):
                """out[128 rows of tile tix, ncol] = lhsT_dram[:, tile].T @ W (+ bias)."""
                ps = psp.tile([128, 512], F32, space="PSUM", tag="mmps")
                have_bias = bias_sb is not None
                if have_bias:
                    nc.tensor.matmul(ps[:, :ncol], ones1[:1, :128],
                                     bias_sb[:1, :ncol], start=True, stop=False)
                for dc in range(4):
                    lh = ldp.tile([128, 128], F32, tag="lhst")
                    nc.sync.dma_start(
                        out=lh[:], in_=lhsT_dram[dc * 128:(dc + 1) * 128,
                                                 tix * 128:(tix + 1) * 128])
                    nc.tensor.matmul(ps[:, :ncol], lh[:],
                                     w_sb[:, dc * ncol:(dc + 1) * ncol],
                                     start=(not have_bias and dc == 0),
                                     stop=(dc == 3))
                if evac == "v":
                    nc.scalar.copy(out_ap, ps[:, :ncol])
                else:  # DRAM destination: stage through SBUF (DMA can't read PSUM)
                    stg = ldp.tile([128, 512], BF16, tag="stgb")
                    nc.scalar.copy(stg[:, :ncol], ps[:, :ncol])
                    nc.sync.dma_start(out=out_ap, in_=stg[:, :ncol])

            Wt_t = load_w(Wt_d)
            for lt in range(NT):
                mm_rows(u_sb[lt][:], tT, lt, Wt_t[:], D, bt_sb)
            WtR_t = load_w(WtR_d)
            for lt in range(NT):
                mm_rows(uR_sb[lt][:], tT, lt, WtR_t[:], D, btR_sb)
                mm_rows(trio[lt][:], tT, lt, Wtr3_sb, 48, None)

            Wc_t = load_w(Wc_d)
            for kt in range(KT):
                mm_rows(Tctx[kt * 128:(kt + 1) * 128, 0:D], cT, kt, Wc_t[:], D,
                        bc_sb, evac="dma")
                mm_rows(Tctx[kt * 128:(kt + 1) * 128, D:D + 32], cT, kt, Wacl_sb, 32,
                        bacl_sb, evac="dma")

            # tpbT: per-octet score bias columns, partition layout p = ls*16 + e
            for lt in range(NT):
                for oc in range(16):
                    nc.sync.dma_start(
                        out=tpbT[:, lt * 16 + oc:lt * 16 + oc + 1],
                        in_=trio[lt][oc * 8:(oc + 1) * 8, 0:EH])

            # cpT [16, K] = Ws1c.T @ context^T + bs1, then replicate 8x on partitions
            cpT_t = medp.tile([EH, K], F32, tag="cpTt")
            cpT = cpT_t[:]
            for nt4 in range(4):
                nsl = slice(nt4 * 512, (nt4 + 1) * 512)
                ps = psp.tile([128, 512], F32, space="PSUM", tag="mmps")
                nc.tensor.matmul(ps[:EH, :], bs1_sb[:1, :], ones1[:1, :512],
                                 start=True, stop=False)
                for dc in range(4):
                    lh = ldp.tile([128, EH], F32, tag="lhst16")
                    nc.sync.dma_start(out=lh[:],
                                      in_=Ws1c_d[dc * 128:(dc + 1) * 128, :])
                    rh = ldp.tile([128, 512], F32, tag="ctchunk")
                    nc.sync.dma_start(out=rh[:], in_=cT[dc * 128:(dc + 1) * 128, nsl])
                    nc.tensor.matmul(ps[:EH, :], lh[:], rh[:],
                                     start=False, stop=(dc == 3))
                nc.vector.tensor_copy(cpT[:, nsl], ps[:EH, :])
            for ls in range(8):
                nc.sync.dma_start(out=cpTrep[ls * 16:(ls + 1) * 16, :], in_=cpT[:, :])

            # ---------- per l-tile ----------
            for lt in range(NT):
                # ----- scores + top-8 -----
                scores = medp.tile([128, K], F32, tag="scores")
                for oc in range(16):
                    for hf in range(2):
                        g_sc = medp.tile([128, K // 2], F32, tag="gsc")
                        nc.scalar.activation(
                            g_sc[:], cpTrep[:, hf * 1024:(hf + 1) * 1024], ACTF.Gelu,
                            bias=tpbT[:, lt * 16 + oc:lt * 16 + oc + 1])
                        for nq in range(2):
                            col = hf * 1024 + nq * 512
                            pssc = psp.tile([8, 512], F32, space="PSUM", tag="small")
                            nc.tensor.matmul(pssc[:, :], Ws2bd_sb[:],
                                             g_sc[:, nq * 512:(nq + 1) * 512],
                                             start=True, stop=True)
                            sstg = medp.tile([8, 512], F32, tag="sstg")
                            nc.vector.tensor_copy(sstg[:], pssc[:, :])
                            nc.sync.dma_start(
                                out=scores[oc * 8:(oc + 1) * 8, col:col + 512],
                                in_=sstg[:])
                mx8 = smlp.tile([128, 8], F32, tag="mx8")
                idx = smlp.tile([128, 8], U32, tag="idx")
                nc.vector.max(out=mx8[:], in_=scores[:])
                nc.vector.max_index(out=idx[:], in_max=mx8[:], in_values=scores[:])

                # ----- gather context-side rows -----
                gb = gbp.tile([128, WWIN * CROW], BF16, tag="gb")
                gbv = gb[:].rearrange("p (w c) -> p w c", w=8)
                for w in range(WWIN):
                    nc.gpsimd.indirect_dma_start(
                        out=gb[:, w * CROW:(w + 1) * CROW],
                        out_offset=None,
                        in_=Tctx[:, :],
                        in_offset=bass.IndirectOffsetOnAxis(ap=idx[:, w:w + 1], axis=0),
                    )

                # ----- per-edge angles -----
                jf = smlp.tile([128, 8], F32, tag="jf")
                nc.vector.tensor_copy(jf[:], idx[:])
                delta = smlp.tile([128, 8], F32, tag="delta")
                nc.vector.tensor_scalar(delta[:], jf[:], lcol_sb[:, lt:lt + 1],
                                        scalar2=None, op0=ALU.subtract)
                ang = medp.tile([128, 8 * 32], F32, tag="ang")
                nc.vector.tensor_tensor(
                    out=ang[:].rearrange("p (w f) -> p w f", w=8),
                    in0=delta[:].unsqueeze(2).to_broadcast((128, 8, 32)),
                    in1=invf_sb[:].unsqueeze(1).to_broadcast((128, 8, 32)),
                    op=ALU.mult)
                # range-reduce to [-pi, pi]: x - 2pi*round(x/2pi), round via
                # the +/- 1.5*2^23 magic-number trick (no mod/floor on DVE ISA)
                MAGIC = 1.5 * 2.0 ** 23
                angt = medp.tile([128, 8 * 32], F32, tag="angt")
                nc.vector.tensor_scalar_mul(angt[:], ang[:], 1.0 / TWO_PI)
                angr = medp.tile([128, 8 * 32], F32, tag="angr")
                nc.vector.tensor_scalar(angr[:], angt[:], MAGIC, scalar2=MAGIC,
                                        op0=ALU.add, op1=ALU.subtract)
                nc.vector.tensor_sub(angt[:], angt[:], angr[:])
                nc.vector.tensor_scalar_mul(ang[:], angt[:], TWO_PI)
                cosb = medp.tile([128, 8 * 32], F32, tag="cosb")
                sinb = medp.tile([128, 8 * 32], F32, tag="sinb")
                nc.scalar.activation(sinb[:], ang[:], ACTF.Sin, scale=-1.0)
                nc.vector.tensor_scalar_mul(angr[:], ang[:], -1.0)
                nc.vector.tensor_max(angt[:], ang[:], angr[:])
                nc.scalar.activation(cosb[:], angt[:], ACTF.Sin, scale=-1.0,
                                     bias=halfpi[:, 0:1])

                # ----- alphas = softplus(gelu(ta+ca) @ Wa2 + ba2) -----
                ha = smlp.tile([128, 8 * EH], F32, tag="ha")
                nc.vector.tensor_tensor(
                    out=ha[:].rearrange("p (w c) -> p w c", w=8),
                    in0=trio[lt][:, 16:32].unsqueeze(1).to_broadcast((128, 8, EH)),
                    in1=gbv[:, :, D:D + EH],
                    op=ALU.add)
                nc.scalar.activation(ha[:], ha[:], ACTF.Gelu)
                haw = smlp.tile([128, 8 * EH], F32, tag="haw")
                nc.vector.tensor_tensor(
                    out=haw[:].rearrange("p (w c) -> p w c", w=8),
                    in0=ha[:].rearrange("p (w c) -> p w c", w=8),
                    in1=wa2_sb[:].unsqueeze(1).to_broadcast((128, 8, EH)),
                    op=ALU.mult)
                alphas = smlp.tile([128, 8], F32, tag="alphas")
                nc.vector.tensor_reduce(alphas[:], haw[:].rearrange(
                    "p (w c) -> p w c", w=8), axis=AX.X, op=ALU.add)
                softplus(alphas[:], alphas[:], ba2_sb[:, 0:1], smlp, "sptmp")

                # ----- g = gelu(tl + cl) and per-w transposes -----
                gmat = smlp.tile([128, 8 * EH], F32, tag="gmat")
                nc.vector.tensor_tensor(
                    out=gmat[:].rearrange("p (w c) -> p w c", w=8),
                    in0=trio[lt][:, 32:48].unsqueeze(1).to_broadcast((128, 8, EH)),
                    in1=gbv[:, :, D + EH:D + 2 * EH],
                    op=ALU.add)
                nc.scalar.activation(gmat[:], gmat[:], ACTF.Gelu)
                gT4 = gbp.tile([128, 2 * 128], BF16, tag="gT4")  # 2 quads side by side
                nc.vector.memset(gT4[:], 0.0)
                for s4 in range(4):  # bias row (constant 1) for the bl2 fold
                    nc.sync.dma_start(out=gT4[32 * s4 + EH:32 * s4 + EH + 1, :],
                                      in_=onesb[:1, 0:256])
                for w in range(WWIN):
                    q, s = w // 4, w % 4
                    pst = psp.tile([EH, 128], F32, space="PSUM", tag="small")
                    nc.tensor.transpose(
                        out=pst[:, :],
                        in_=gmat[:].rearrange("p (w c) -> p w c", w=8)[:, w, :],
                        identity=ident)
                    nc.vector.tensor_copy(
                        gT4[32 * s:32 * s + EH, q * 128:(q + 1) * 128], pst[:, :])

                # ----- per-head loop -----
                for h in range(H):
                    # Lam layout: (w, r9, d) bf16, contiguous
                    Lam = lamp.tile([128, WWIN * R9 * HD], BF16, tag="lam")
                    for w in range(WWIN):
                        q, s = w // 4, w % 4
                        psl = ps4p.tile([128, 512], F32, space="PSUM", tag="lamps")
                        nc.tensor.matmul(
                            psl[:, :], gT4[32 * s:32 * s + 32, q * 128:(q + 1) * 128],
                            Wl2_sb[32 * s:32 * s + 32, h * R * HD:(h + 1) * R * HD],
                            start=True, stop=True, tile_position=(32 * s, 0))
                        nc.scalar.copy(
                            Lam[:, w * R9 * HD:w * R9 * HD + R * HD], psl[:, :])
                    lam4 = Lam[:].rearrange("p (w r d) -> p w r d", w=8, r=R9)
                    # squared row norms -> scale 1/max(norm,1e-12)^2 (square on ACT)
                    n2 = smlp.tile([128, WWIN * R], F32, tag="n2")
                    for w in range(WWIN):
                        sqw = medp.tile([128, R * HD], F32, tag="sqw")
                        nc.scalar.activation(sqw[:], lam4[:, w, 0:R, :], ACTF.Square)
                        nc.vector.tensor_reduce(
                            n2[:].rearrange("p (w r) -> p w r", w=8)[:, w, :],
                            sqw[:].rearrange("p (r d) -> p r d", r=R),
                            axis=AX.X, op=ALU.add)
                    nrm = smlp.tile([128, WWIN * R], F32, tag="nrm")
                    nc.vector.tensor_scalar_max(nrm[:], n2[:], 1e-24)
                    rec9 = smlp.tile([128, WWIN * R9], F32, tag="rec9")
                    nc.vector.memset(rec9[:], 0.0)
                    nc.vector.reciprocal(
                        rec9[:].rearrange("p (w r) -> p w r", w=8, r=R9)[:, :, 0:R],
                        nrm[:].rearrange("p (w r) -> p w r", w=8))

                    usl = u_sb[lt][:, h * HD:(h + 1) * HD]
                    uRsl = uR_sb[lt][:, h * HD:(h + 1) * HD]
                    for t in range(T):
                        stc = slice(t * NT + lt, t * NT + lt + 1)
                        # diff = u_i*cos + uR_i*sin - v_j     [128, (w,d)]
                        diff = medp.tile([128, WWIN * HD], BF16, tag="diff")
                        d3 = diff[:].rearrange("p (w d) -> p w d", w=8)
                        t0 = medp.tile([128, WWIN * HD], BF16, tag="t0")
                        nc.vector.tensor_tensor(
                            out=t0[:].rearrange("p (w a b) -> p w a b", w=8, a=2),
                            in0=usl.rearrange("p (a b) -> p a b", a=2)
                                .unsqueeze(1).to_broadcast((128, 8, 2, 32)),
                            in1=cosb[:].rearrange("p (w f) -> p w f", w=8)
                                .unsqueeze(2).to_broadcast((128, 8, 2, 32)),
                            op=ALU.mult)
                        t1 = medp.tile([128, WWIN * HD], BF16, tag="t0")
                        nc.vector.tensor_tensor(
                            out=t1[:].rearrange("p (w a b) -> p w a b", w=8, a=2),
                            in0=uRsl.rearrange("p (a b) -> p a b", a=2)
                                .unsqueeze(1).to_broadcast((128, 8, 2, 32)),
                            in1=sinb[:].rearrange("p (w f) -> p w f", w=8)
                                .unsqueeze(2).to_broadcast((128, 8, 2, 32)),
                            op=ALU.mult)
                        nc.vector.tensor_tensor(out=t0[:], in0=t0[:], in1=t1[:],
                                                op=ALU.add)
                        nc.vector.tensor_tensor(
                            out=d3,
                            in0=t0[:].rearrange("p (w d) -> p w d", w=8),
                            in1=gbv[:, :, h * HD:(h + 1) * HD],
                            op=ALU.subtract)
                        # alpha slot: Lam[:, :, 8, :] = diff
                        nc.vector.tensor_copy(lam4[:, :, R:R9, :].squeeze(2), d3)
                        # einsum1: Ld[w,r] = sum_d Lam*diff
                        prod = prodp.tile([128, WWIN * R9 * HD], BF16, tag="prodb")
                        nc.vector.tensor_tensor(
                            out=prod[:].rearrange("p (w r d) -> p w r d", w=8, r=R9),
                            in0=lam4,
                            in1=d3.unsqueeze(2).to_broadcast((128, 8, R9, HD)),
                            op=ALU.mult)
                        ld = smlp.tile([128, WWIN * R9], F32, tag="ld")
                        nc.vector.tensor_reduce(
                            ld[:].rearrange("p (w r) -> p w r", w=8),
                            prod[:].rearrange("p (w r d) -> p w r d", w=8, r=R9),
                            axis=AX.X, op=ALU.add)
                        ld2 = smlp.tile([128, WWIN * R9], F32, tag="ld2")
                        nc.vector.tensor_tensor(ld2[:], ld[:], rec9[:], op=ALU.mult)
                        nc.vector.tensor_copy(
                            ld2[:].rearrange("p (w r) -> p w r", w=8)[:, :, R:R9]
                            .squeeze(2), alphas[:])
                        # einsum2 (+alpha*diff via slot 8): res[w,d] = sum_r9 Ld2*Lam
                        # einsum2 products on gpsimd, split in w-halves so the
                        # DVE reduce of half 1 overlaps gpsimd's half 2
                        prod2 = prodp.tile([128, WWIN * R9 * HD], BF16, tag="prodb2")
                        p2v = prod2[:].rearrange("p (w d r) -> p w d r", w=8, d=HD)
                        lamT = lam4.transpose([0, 1, 3, 2])
                        ld2v = ld2[:].rearrange("p (w r) -> p w r", w=8)\
                            .unsqueeze(2).to_broadcast((128, 8, HD, R9))
                        res = medp.tile([128, WWIN * HD], F32, tag="res")
                        resv = res[:].rearrange("p (w d) -> p w d", w=8)
                        for hw_ in range(2):
                            wsl = slice(hw_ * 4, hw_ * 4 + 4)
                            nc.gpsimd.tensor_tensor(
                                out=p2v[:, wsl], in0=lamT[:, wsl],
                                in1=ld2v[:, wsl], op=ALU.mult)
                            nc.vector.tensor_reduce(
                                resv[:, wsl], p2v[:, wsl], axis=AX.X, op=ALU.add)
                        # scatter: sum over w
                        rsum = smlp.tile([128, HD], F32, tag="rsum")
                        nc.vector.tensor_reduce(
                            rsum[:],
                            res[:].rearrange("p (w d) -> p d w", w=8),
                            axis=AX.X, op=ALU.add)
                        # u -= step*rsum ; uR via rotate_half identity
                        nc.vector.scalar_tensor_tensor(
                            out=usl, in0=rsum[:], scalar=stpn_sb[:, stc],
                            in1=usl, op0=ALU.mult, op1=ALU.add)
                        nc.vector.scalar_tensor_tensor(
                            out=uRsl[:, 0:32], in0=rsum[:, 32:64],
                            scalar=stp_sb[:, stc], in1=uRsl[:, 0:32],
                            op0=ALU.mult, op1=ALU.add)
                        nc.vector.scalar_tensor_tensor(
                            out=uRsl[:, 32:64], in0=rsum[:, 0:32],
                            scalar=stpn_sb[:, stc], in1=uRsl[:, 32:64],
                            op0=ALU.mult, op1=ALU.add)

            # ---------- output projection: y = u @ Wo + bo ----------
            Wo_t = load_w(Wo_d)
            for lt in range(NT):
                psy = psp.tile([128, 512], F32, space="PSUM", tag="mmps")
                nc.tensor.matmul(psy[:, :], ones1[:1, :128], bo_sb[:1, :],
                                 start=True, stop=False)
                for dc in range(4):
                    pst = psp.tile([128, 128], F32, space="PSUM", tag="small")
                    nc.tensor.transpose(
                        out=pst[:, :], in_=u_sb[lt][:, dc * 128:(dc + 1) * 128],
                        identity=ident)
                    uT = ldp.tile([128, 128], F32, tag="uT")
                    nc.vector.tensor_copy(uT[:], pst[:, :])
                    nc.tensor.matmul(psy[:, :], uT[:], Wo_t[:, dc * D:(dc + 1) * D],
                                     start=False, stop=(dc == 3))
                ystg = ldp.tile([128, 512], F32, tag="stg")
                nc.vector.tensor_copy(ystg[:], psy[:, :])
                nc.sync.dma_start(out=y_d[lt * 128:(lt + 1) * 128, :], in_=ystg[:])

    nc.finalize()
    return nc


def _rot_cols(Wm):
    """Fold rotate_half into output columns: out cols = [-cols(h, hi), cols(h, lo)]."""
    W4 = Wm.reshape(-1, H, 2, HD // 2)
    out = np.concatenate([-W4[:, :, 1], W4[:, :, 0]], axis=2)
    return np.ascontiguousarray(out.reshape(Wm.shape))


def make_in_maps(inputs):
    """Host-side prep: slice/transpose inputs into the 8 per-core input maps."""
    target = np.asarray(inputs["target"], np.float32)
    context = np.asarray(inputs["context"], np.float32)
    Wt = np.asarray(inputs["Wt"], np.float32)
    bt = np.asarray(inputs["bt"], np.float32)
    Wc = np.asarray(inputs["Wc"], np.float32)
    bc = np.asarray(inputs["bc"], np.float32)
    Ws1 = np.asarray(inputs["Ws1"], np.float32)
    bs1 = np.asarray(inputs["bs1"], np.float32)
    Ws2 = np.asarray(inputs["Ws2"], np.float32)
    Wa1 = np.asarray(inputs["Wa1"], np.float32)
    ba1 = np.asarray(inputs["ba1"], np.float32)
    Wa2 = np.asarray(inputs["Wa2"], np.float32)
    ba2 = np.asarray(inputs["ba2"], np.float32)
    Wl1 = np.asarray(inputs["Wl1"], np.float32)
    bl1 = np.asarray(inputs["bl1"], np.float32)
    Wl2 = np.asarray(inputs["Wl2"], np.float32)
    bl2 = np.asarray(inputs["bl2"], np.float32)
    step_sizes = np.asarray(inputs["step_sizes"], np.float32)
    Wo = np.asarray(inputs["Wo"], np.float32)
    bo = np.asarray(inputs["bo"], np.float32)


    import ml_dtypes
    Ws2bd = np.zeros((128, 8), np.float32)
    for ls in range(8):
        Ws2bd[ls * 16:(ls + 1) * 16, ls] = Ws2[:, 0]
    Wl2s = np.zeros((128, H * R * HD), np.float32)
    for s in range(4):
        Wl2s[32 * s:32 * s + EH, :] = Wl2
        Wl2s[32 * s + EH, :] = bl2
    Wl2s = Wl2s.astype(ml_dtypes.bfloat16)
    invf = (1.0 / (10000.0 ** (np.arange(0, HD, 2, dtype=np.float32) / HD)))[None, :]

    common = dict(
        Wt=Wt, WtR=_rot_cols(Wt), Wc=Wc, Wo=Wo,
        bt=bt[None, :], btR=_rot_cols(bt[None, :]), bc=bc[None, :], bo=bo[None, :],
        Wtr3=np.ascontiguousarray(np.concatenate([Ws1[:D], Wa1[:D], Wl1[:D]], axis=1)),
        Ws1c=np.ascontiguousarray(Ws1[D:]),
        Wacl=np.ascontiguousarray(np.concatenate([Wa1[D:], Wl1[D:]], axis=1)),
        bs1=bs1[None, :],
        bacl=np.concatenate([ba1, bl1])[None, :],
        Ws2bd=Ws2bd, Wa2=np.ascontiguousarray(Wa2.T),
        ba2=np.asarray(ba2, np.float32).reshape(1, 1), Wl2=Wl2s,
        invf=np.ascontiguousarray(invf, np.float32),
    )

    in_maps = []
    for c in range(8):
        b, rc = c // 4, c % 4
        rows = slice(rc * LC, (rc + 1) * LC)
        stp = np.ascontiguousarray(
            step_sizes[:, rows].reshape(T, NT, 128).transpose(2, 0, 1)
            .reshape(128, T * NT))
        lcol = np.ascontiguousarray(
            (rc * LC + np.arange(LC, dtype=np.float32)).reshape(NT, 128).T)
        m = dict(common)
        m.update(
            tT=np.ascontiguousarray(target[b, rows].T),
            cT=np.ascontiguousarray(context[b].T),
            stp=stp, lcol=lcol,
        )
        in_maps.append(m)
    return in_maps


_NC_CACHE = {}


def kernel(**inputs):
    if "nc" not in _NC_CACHE:
        _NC_CACHE["nc"] = build_program()
    nc = _NC_CACHE["nc"]
    in_maps = make_in_maps(inputs)
    res = run_bass_kernel_spmd(nc, in_maps, list(range(8)))
    out = np.empty((B, L, D), np.float32)
    for c in range(8):
        b, rc = c // 4, c % 4
        out[b, rc * LC:(rc + 1) * LC] = res.results[c]["y"]
    return out



# revision 6
# speedup vs baseline: 1.1485x; 1.1485x over previous
"""CrossConsensus kernel for 8 Trainium2 NeuronCores.

Sharding: data-parallel over B*L rows. Core c handles batch b=c//4,
target rows [ (c%4)*512, (c%4+1)*512 ).  All computation is row-local
(edge_i = repeat(arange(L), 8) means each edge scatters back to its own
source row), so there are no collectives; each core needs its target
row-chunk plus the full context of its batch.

v2: r8 einsums (alpha folded via a per-l-tile alpharep instead of the
9th rank slot), einsum1 product on GPSIMD, single-op einsum2 product on
GPSIMD, fused across-w norms, ACT-engine PSUM evacuations, and
head-pair software pipelining to keep DVE fed while GPSIMD works.
"""

import math

import numpy as np

import concourse.bass as bass
import concourse.bacc as bacc
import concourse.tile as tile
from concourse import mybir
from concourse.bass_utils import run_bass_kernel_spmd
from concourse.masks import make_identity

F32 = mybir.dt.float32
BF16 = mybir.dt.bfloat16
U32 = mybir.dt.uint32
AX = mybir.AxisListType
ALU = mybir.AluOpType
ACTF = mybir.ActivationFunctionType

# problem constants (hardcoded per the harness contract)
B, L, K, D = 2, 2048, 2048, 512
H, R, WWIN, T, EH = 8, 8, 8, 2, 16
HD = D // H            # 64
LC = L * B // 8        # 512 rows per core
NT = LC // 128         # 4 l-tiles per core
KT = K // 128          # 16 k-tiles
CROW = D + 2 * EH      # 544: gather-table row [v(512) | ca(16) | cl(16)]
TWO_PI = 2.0 * math.pi


def build_program():
    nc = bacc.Bacc()

    # ---------------- external I/O ----------------
    tT = nc.dram_tensor("tT", [D, LC], F32, kind="ExternalInput")        # target^T
    cT = nc.dram_tensor("cT", [D, K], F32, kind="ExternalInput")         # context^T
    Wt_d = nc.dram_tensor("Wt", [D, D], F32, kind="ExternalInput")
    WtR_d = nc.dram_tensor("WtR", [D, D], F32, kind="ExternalInput")     # rotate_half-folded
    Wc_d = nc.dram_tensor("Wc", [D, D], F32, kind="ExternalInput")
    Wo_d = nc.dram_tensor("Wo", [D, D], F32, kind="ExternalInput")
    bt_d = nc.dram_tensor("bt", [1, D], F32, kind="ExternalInput")
    btR_d = nc.dram_tensor("btR", [1, D], F32, kind="ExternalInput")
    bc_d = nc.dram_tensor("bc", [1, D], F32, kind="ExternalInput")
    bo_d = nc.dram_tensor("bo", [1, D], F32, kind="ExternalInput")
    Wtr3_d = nc.dram_tensor("Wtr3", [D, 48], F32, kind="ExternalInput")  # [Ws1t|Wa1t|Wl1t]
    Ws1c_d = nc.dram_tensor("Ws1c", [D, EH], F32, kind="ExternalInput")
    Wacl_d = nc.dram_tensor("Wacl", [D, 32], F32, kind="ExternalInput")  # [Wa1c|Wl1c]
    bs1_d = nc.dram_tensor("bs1", [1, EH], F32, kind="ExternalInput")
    bacl_d = nc.dram_tensor("bacl", [1, 32], F32, kind="ExternalInput")  # [ba1|bl1]
    Ws2bd_d = nc.dram_tensor("Ws2bd", [128, 8], F32, kind="ExternalInput")
    Wa2_d = nc.dram_tensor("Wa2", [1, EH], F32, kind="ExternalInput")
    ba2_d = nc.dram_tensor("ba2", [1, 1], F32, kind="ExternalInput")
    Wl2_d = nc.dram_tensor("Wl2", [128, H * R * HD], BF16, kind="ExternalInput")
    stp_d = nc.dram_tensor("stp", [128, T * NT], F32, kind="ExternalInput")
    lcol_d = nc.dram_tensor("lcol", [128, NT], F32, kind="ExternalInput")
    invf_d = nc.dram_tensor("invf", [1, HD // 2], F32, kind="ExternalInput")
    y_d = nc.dram_tensor("y", [LC, D], F32, kind="ExternalOutput")

    # internal DRAM gather table
    Tctx = nc.dram_tensor("Tctx", [K, CROW], BF16)

    # ---------------- persistent SBUF (static allocs, before pools) ----------
    ident = nc.alloc_sbuf_tensor("ident", [128, 128], F32).ap()
    ones1 = nc.alloc_sbuf_tensor("ones1", [1, 512], F32).ap()
    u_sb = [nc.alloc_sbuf_tensor(f"u{i}", [128, D], F32).ap() for i in range(NT)]
    uR_sb = [nc.alloc_sbuf_tensor(f"uR{i}", [128, D], F32).ap() for i in range(NT)]
    trio = [nc.alloc_sbuf_tensor(f"trio{i}", [128, 48], F32).ap() for i in range(NT)]
    Wl2_sb = nc.alloc_sbuf_tensor("Wl2sb", [128, H * R * HD], BF16).ap()
    cpTrep = nc.alloc_sbuf_tensor("cpTrep", [128, K], F32).ap()
    tpbT = nc.alloc_sbuf_tensor("tpbT", [128, NT * 16], F32).ap()
    invf_sb = nc.alloc_sbuf_tensor("invfsb", [128, HD // 2], F32).ap()
    wa2_sb = nc.alloc_sbuf_tensor("wa2sb", [128, EH], F32).ap()
    ba2_sb = nc.alloc_sbuf_tensor("ba2sb", [128, 1], F32).ap()
    stp_sb = nc.alloc_sbuf_tensor("stpsb", [128, T * NT], F32).ap()
    stpn_sb = nc.alloc_sbuf_tensor("stpnsb", [128, T * NT], F32).ap()
    lcol_sb = nc.alloc_sbuf_tensor("lcolsb", [128, NT], F32).ap()
    bs1_sb = nc.alloc_sbuf_tensor("bs1sb", [1, EH], F32).ap()
    bacl_sb = nc.alloc_sbuf_tensor("baclsb", [1, 32], F32).ap()
    bt_sb = nc.alloc_sbuf_tensor("btsb", [1, D], F32).ap()
    btR_sb = nc.alloc_sbuf_tensor("btRsb", [1, D], F32).ap()
    bc_sb = nc.alloc_sbuf_tensor("bcsb", [1, D], F32).ap()
    bo_sb = nc.alloc_sbuf_tensor("bosb", [1, D], F32).ap()
    Ws2bd_sb = nc.alloc_sbuf_tensor("ws2bdsb", [128, 8], F32).ap()
    Wtr3_sb = nc.alloc_sbuf_tensor("wtr3sb", [128, 4 * 48], F32).ap()
    Wacl_sb = nc.alloc_sbuf_tensor("waclsb", [128, 4 * 32], F32).ap()
    halfpi = nc.alloc_sbuf_tensor("halfpi", [128, 1], F32).ap()
    onec = nc.alloc_sbuf_tensor("onec", [128, 1], F32).ap()
    onesb = nc.alloc_sbuf_tensor("onesb", [1, 256], BF16).ap()

    with tile.TileContext(nc) as tc:
        with (
            tc.tile_pool(name="ld", bufs=2) as ldp,             # small staging tiles
            tc.tile_pool(name="gbp", bufs=2) as gbp,            # gather block
            tc.tile_pool(name="lamp", bufs=2) as lamp,          # Lam
            tc.tile_pool(name="prodp", bufs=2) as prodp,        # einsum products
            tc.tile_pool(name="med", bufs=2) as medp,
            tc.tile_pool(name="sml", bufs=2) as smlp,
            tc.tile_pool(name="wp", bufs=1) as wp,
            tc.tile_pool(name="ps", bufs=2, space="PSUM") as psp,
            tc.tile_pool(name="ps4", bufs=4, space="PSUM") as ps4p,
        ):
            # ---------- constants ----------
            make_identity(nc, ident)
            nc.vector.memset(ones1, 1.0)
            nc.vector.memset(halfpi, math.pi / 2)
            nc.vector.memset(onec, 1.0)
            nc.vector.memset(onesb, 1.0)
            nc.sync.dma_start(out=invf_sb, in_=invf_d[:].partition_broadcast(128))
            nc.sync.dma_start(out=wa2_sb, in_=Wa2_d[:].partition_broadcast(128))
            nc.sync.dma_start(out=ba2_sb, in_=ba2_d[:].partition_broadcast(128))
            nc.sync.dma_start(out=lcol_sb, in_=lcol_d[:])
            nc.sync.dma_start(out=bs1_sb, in_=bs1_d[:])
            nc.sync.dma_start(out=bacl_sb, in_=bacl_d[:])
            nc.sync.dma_start(out=bt_sb, in_=bt_d[:])
            nc.sync.dma_start(out=btR_sb, in_=btR_d[:])
            nc.sync.dma_start(out=bc_sb, in_=bc_d[:])
            nc.sync.dma_start(out=bo_sb, in_=bo_d[:])
            nc.sync.dma_start(out=Ws2bd_sb, in_=Ws2bd_d[:])
            nc.sync.dma_start(out=Wl2_sb, in_=Wl2_d[:])
            for dc in range(4):
                sl = slice(dc * 128, (dc + 1) * 128)
                nc.sync.dma_start(out=Wtr3_sb[:, dc * 48:(dc + 1) * 48], in_=Wtr3_d[sl, :])
                nc.sync.dma_start(out=Wacl_sb[:, dc * 32:(dc + 1) * 32], in_=Wacl_d[sl, :])

            def load_w(dram):
                t = wp.tile([128, 4 * D], F32, tag="wrhs")
                for dc in range(4):
                    nc.sync.dma_start(out=t[:, dc * D:(dc + 1) * D],
                                      in_=dram[dc * 128:(dc + 1) * 128, :])
                return t

            def softplus(dst, src, bias_ap, tmp_pool, tmp_tag):
                """dst = softplus(src + bias) = relu(x) + ln(1+exp(-|x|)).
                No softplus HW table; composed from abs/exp/ln (one table set)."""
                shp = [src.shape[0], src.free_size()]
                a = tmp_pool.tile(shp, F32, tag=tmp_tag)
                if bias_ap is None:
                    nc.scalar.activation(a[:], src, ACTF.Abs)
                    nc.vector.tensor_scalar(dst, src, 0.0, scalar2=None, op0=ALU.max)
                else:
                    nc.scalar.activation(a[:], src, ACTF.Abs, bias=bias_ap)
                    nc.vector.tensor_scalar(dst, src, bias_ap, scalar2=0.0,
                                            op0=ALU.add, op1=ALU.max)
                nc.scalar.activation(a[:], a[:], ACTF.Exp, scale=-1.0)
                nc.scalar.activation(a[:], a[:], ACTF.Ln, bias=onec[:, 0:1])
                nc.vector.tensor_tensor(dst, dst, a[:], op=ALU.add)

            stp_raw = smlp.tile([128, T * NT], F32, tag="stpraw")
            nc.sync.dma_start(out=stp_raw[:], in_=stp_d[:])
            softplus(stp_sb, stp_raw[:], None, smlp, "sptmp")
            nc.vector.tensor_scalar_mul(stpn_sb, stp_sb, -1.0)

            # ---------- dense projections ----------
            def mm_rows(out_ap, lhsT_dram, tix, w_sb, ncol, bias_sb, evac="v"):
                """out[128 rows of tile tix, ncol] = lhsT_dram[:, tile].T @ W (+ bias)."""
                ps = psp.tile([128, 512], F32, space="PSUM", tag="mmps")
                have_bias = bias_sb is not None
                if have_bias:
                    nc.tensor.matmul(ps[:, :ncol], ones1[:1, :128],
                                     bias_sb[:1, :ncol], start=True, stop=False)
                for dc in range(4):
                    lh = ldp.tile([128, 128], F32, tag="lhst")
                    nc.sync.dma_start(
                        out=lh[:], in_=lhsT_dram[dc * 128:(dc + 1) * 128,
                                                 tix * 128:(tix + 1) * 128])
                    nc.tensor.matmul(ps[:, :ncol], lh[:],
                                     w_sb[:, dc * ncol:(dc + 1) * ncol],
                                     start=(not have_bias and dc == 0),
                                     stop=(dc == 3))
                if evac == "v":
                    nc.scalar.copy(out_ap, ps[:, :ncol])
                else:  # DRAM destination: stage through SBUF (DMA can't read PSUM)
                    stg = ldp.tile([128, 512], BF16, tag="stgb")
                    nc.scalar.copy(stg[:, :ncol], ps[:, :ncol])
                    nc.sync.dma_start(out=out_ap, in_=stg[:, :ncol])

            Wt_t = load_w(Wt_d)
            for lt in range(NT):
                mm_rows(u_sb[lt][:], tT, lt, Wt_t[:], D, bt_sb)
            WtR_t = load_w(WtR_d)
            for lt in range(NT):
                mm_rows(uR_sb[lt][:], tT, lt, WtR_t[:], D, btR_sb)
                mm_rows(trio[lt][:], tT, lt, Wtr3_sb, 48, None)

            Wc_t = load_w(Wc_d)
            for kt in range(KT):
                mm_rows(Tctx[kt * 128:(kt + 1) * 128, 0:D], cT, kt, Wc_t[:], D,
                        bc_sb, evac="dma")
                mm_rows(Tctx[kt * 128:(kt + 1) * 128, D:D + 32], cT, kt, Wacl_sb, 32,
                        bacl_sb, evac="dma")

            # tpbT: per-octet score bias columns, partition layout p = ls*16 + e
            for lt in range(NT):
                for oc in range(16):
                    nc.sync.dma_start(
                        out=tpbT[:, lt * 16 + oc:lt * 16 + oc + 1],
                        in_=trio[lt][oc * 8:(oc + 1) * 8, 0:EH])

            # cpT [16, K] = Ws1c.T @ context^T + bs1, then replicate 8x on partitions
            cpT_t = medp.tile([EH, K], F32, tag="cpTt", bufs=1)
            cpT = cpT_t[:]
            for nt4 in range(4):
                nsl = slice(nt4 * 512, (nt4 + 1) * 512)
                ps = psp.tile([128, 512], F32, space="PSUM", tag="mmps")
                nc.tensor.matmul(ps[:EH, :], bs1_sb[:1, :], ones1[:1, :512],
                                 start=True, stop=False)
                for dc in range(4):
                    lh = ldp.tile([128, EH], F32, tag="lhst16")
                    nc.sync.dma_start(out=lh[:],
                                      in_=Ws1c_d[dc * 128:(dc + 1) * 128, :])
                    rh = ldp.tile([128, 512], F32, tag="ctchunk")
                    nc.sync.dma_start(out=rh[:], in_=cT[dc * 128:(dc + 1) * 128, nsl])
                    nc.tensor.matmul(ps[:EH, :], lh[:], rh[:],
                                     start=False, stop=(dc == 3))
                nc.vector.tensor_copy(cpT[:, nsl], ps[:EH, :])
            for ls in range(8):
                nc.sync.dma_start(out=cpTrep[ls * 16:(ls + 1) * 16, :], in_=cpT[:, :])

            # ---------- per l-tile ----------
            for lt in range(NT):
                # ----- scores + top-8 -----
                scores = medp.tile([128, K], F32, tag="scores", bufs=1)
                for oc in range(16):
                    for hf in range(2):
                        g_sc = medp.tile([128, K // 2], F32, tag="gsc")
                        nc.scalar.activation(
                            g_sc[:], cpTrep[:, hf * 1024:(hf + 1) * 1024], ACTF.Gelu,
                            bias=tpbT[:, lt * 16 + oc:lt * 16 + oc + 1])
                        for nq in range(2):
                            col = hf * 1024 + nq * 512
                            pssc = psp.tile([8, 512], F32, space="PSUM", tag="small")
                            nc.tensor.matmul(pssc[:, :], Ws2bd_sb[:],
                                             g_sc[:, nq * 512:(nq + 1) * 512],
                                             start=True, stop=True)
                            sstg = medp.tile([8, 512], F32, tag="sstg")
                            nc.scalar.copy(sstg[:], pssc[:, :])
                            nc.sync.dma_start(
                                out=scores[oc * 8:(oc + 1) * 8, col:col + 512],
                                in_=sstg[:])
                mx8 = smlp.tile([128, 8], F32, tag="mx8")
                idx = smlp.tile([128, 8], U32, tag="idx")
                nc.vector.max(out=mx8[:], in_=scores[:])
                nc.vector.max_index(out=idx[:], in_max=mx8[:], in_values=scores[:])

                # ----- gather context-side rows -----
                gb = gbp.tile([128, WWIN * CROW], BF16, tag="gb")
                gbv = gb[:].rearrange("p (w c) -> p w c", w=8)
                for w in range(WWIN):
                    nc.gpsimd.indirect_dma_start(
                        out=gb[:, w * CROW:(w + 1) * CROW],
                        out_offset=None,
                        in_=Tctx[:, :],
                        in_offset=bass.IndirectOffsetOnAxis(ap=idx[:, w:w + 1], axis=0),
                    )

                # ----- per-edge angles -----
                jf = smlp.tile([128, 8], F32, tag="jf")
                nc.vector.tensor_copy(jf[:], idx[:])
                delta = smlp.tile([128, 8], F32, tag="delta")
                nc.vector.tensor_scalar(delta[:], jf[:], lcol_sb[:, lt:lt + 1],
                                        scalar2=None, op0=ALU.subtract)
                ang = medp.tile([128, 8 * 32], F32, tag="ang")
                nc.vector.tensor_tensor(
                    out=ang[:].rearrange("p (w f) -> p w f", w=8),
                    in0=delta[:].unsqueeze(2).to_broadcast((128, 8, 32)),
                    in1=invf_sb[:].unsqueeze(1).to_broadcast((128, 8, 32)),
                    op=ALU.mult)
                # range-reduce to [-pi, pi]: x - 2pi*round(x/2pi), round via
                # the +/- 1.5*2^23 magic-number trick (no mod/floor on DVE ISA)
                MAGIC = 1.5 * 2.0 ** 23
                angt = medp.tile([128, 8 * 32], F32, tag="angt")
                nc.vector.tensor_scalar_mul(angt[:], ang[:], 1.0 / TWO_PI)
                angr = medp.tile([128, 8 * 32], F32, tag="angr")
                nc.vector.tensor_scalar(angr[:], angt[:], MAGIC, scalar2=MAGIC,
                                        op0=ALU.add, op1=ALU.subtract)
                nc.vector.tensor_sub(angt[:], angt[:], angr[:])
                nc.vector.tensor_scalar_mul(ang[:], angt[:], TWO_PI)
                cosb = medp.tile([128, 8 * 32], F32, tag="cosb")
                sinb = medp.tile([128, 8 * 32], F32, tag="sinb")
                nc.scalar.activation(sinb[:], ang[:], ACTF.Sin, scale=-1.0)
                nc.vector.tensor_scalar_mul(angr[:], ang[:], -1.0)
                nc.vector.tensor_max(angt[:], ang[:], angr[:])
                nc.scalar.activation(cosb[:], angt[:], ACTF.Sin, scale=-1.0,
                                     bias=halfpi[:, 0:1])

                # ----- alphas = softplus(gelu(ta+ca) @ Wa2 + ba2) -----
                ha = smlp.tile([128, 8 * EH], F32, tag="ha")
                nc.vector.tensor_tensor(
                    out=ha[:].rearrange("p (w c) -> p w c", w=8),
                    in0=trio[lt][:, 16:32].unsqueeze(1).to_broadcast((128, 8, EH)),
                    in1=gbv[:, :, D:D + EH],
                    op=ALU.add)
                nc.scalar.activation(ha[:], ha[:], ACTF.Gelu)
                haw = smlp.tile([128, 8 * EH], F32, tag="haw")
                nc.vector.tensor_tensor(
                    out=haw[:].rearrange("p (w c) -> p w c", w=8),
                    in0=ha[:].rearrange("p (w c) -> p w c", w=8),
                    in1=wa2_sb[:].unsqueeze(1).to_broadcast((128, 8, EH)),
                    op=ALU.mult)
                alphas = smlp.tile([128, 8], F32, tag="alphas")
                nc.vector.tensor_reduce(alphas[:], haw[:].rearrange(
                    "p (w c) -> p w c", w=8), axis=AX.X, op=ALU.add)
                softplus(alphas[:], alphas[:], ba2_sb[:, 0:1], smlp, "sptmp")
                # alpharep [128, (w d)] = alphas replicated over d (t/h-invariant)
                alpharep = smlp.tile([128, WWIN * HD], F32, tag="alpharep")
                nc.vector.tensor_copy(
                    alpharep[:].rearrange("p (w d) -> p w d", w=8),
                    alphas[:].unsqueeze(2).to_broadcast((128, 8, HD)))

                # ----- g = gelu(tl + cl) and per-w transposes -----
                gmat = smlp.tile([128, 8 * EH], F32, tag="gmat")
                nc.vector.tensor_tensor(
                    out=gmat[:].rearrange("p (w c) -> p w c", w=8),
                    in0=trio[lt][:, 32:48].unsqueeze(1).to_broadcast((128, 8, EH)),
                    in1=gbv[:, :, D + EH:D + 2 * EH],
                    op=ALU.add)
                nc.scalar.activation(gmat[:], gmat[:], ACTF.Gelu)
                gT4 = gbp.tile([128, 2 * 128], BF16, tag="gT4")  # 2 quads side by side
                nc.vector.memset(gT4[:], 0.0)
                for s4 in range(4):  # bias row (constant 1) for the bl2 fold
                    nc.sync.dma_start(out=gT4[32 * s4 + EH:32 * s4 + EH + 1, :],
                                      in_=onesb[:1, 0:256])
                for w in range(WWIN):
                    q, s = w // 4, w % 4
                    pst = psp.tile([EH, 128], F32, space="PSUM", tag="small")
                    nc.tensor.transpose(
                        out=pst[:, :],
                        in_=gmat[:].rearrange("p (w c) -> p w c", w=8)[:, w, :],
                        identity=ident)
                    nc.vector.tensor_copy(
                        gT4[32 * s:32 * s + EH, q * 128:(q + 1) * 128], pst[:, :])

                # ----- per-head-pair pipelined t-loop -----
                for hp in range(H // 2):
                    pair = (2 * hp, 2 * hp + 1)
                    lam_h = {}
                    rec_h = {}
                    for h in pair:
                        # Lam layout: (w, r, d) bf16, contiguous
                        Lam = lamp.tile([128, WWIN * R * HD], BF16, tag="lam")
                        lam_h[h] = Lam
                        for w in range(WWIN):
                            q, s = w // 4, w % 4
                            psl = ps4p.tile([128, 512], F32, space="PSUM", tag="lamps")
                            nc.tensor.matmul(
                                psl[:, :], gT4[32 * s:32 * s + 32, q * 128:(q + 1) * 128],
                                Wl2_sb[32 * s:32 * s + 32, h * R * HD:(h + 1) * R * HD],
                                start=True, stop=True, tile_position=(32 * s, 0))
                            nc.scalar.copy(
                                Lam[:, w * R * HD:(w + 1) * R * HD], psl[:, :])
                        # squared row norms, fused across w: sq = Lam^2 (ACT)
                        sq = prodp.tile([128, WWIN * R * HD], BF16, tag="sq",
                                        bufs=1)
                        nc.scalar.activation(sq[:], Lam[:], ACTF.Square)
                        n2 = smlp.tile([128, WWIN * R], F32, tag="n2")
                        nc.vector.tensor_reduce(
                            n2[:].rearrange("p (w r) -> p w r", w=8),
                            sq[:].rearrange("p (w r d) -> p w r d", w=8, r=R),
                            axis=AX.X, op=ALU.add)
                        nrm = smlp.tile([128, WWIN * R], F32, tag="nrm")
                        nc.vector.tensor_scalar_max(nrm[:], n2[:], 1e-24)
                        rec = smlp.tile([128, WWIN * R], F32, tag="rec")
                        rec_h[h] = rec
                        nc.vector.reciprocal(rec[:], nrm[:])

                    usl = {h: u_sb[lt][:, h * HD:(h + 1) * HD] for h in pair}
                    uRsl = {h: uR_sb[lt][:, h * HD:(h + 1) * HD] for h in pair}
                    for t in range(T):
                        stc = slice(t * NT + lt, t * NT + lt + 1)
                        diff_h = {}
                        prod_h = {}
                        for h in pair:
                            lam4 = lam_h[h][:].rearrange(
                                "p (w r d) -> p w r d", w=8, r=R)
                            # diff = u_i*cos + uR_i*sin - v_j     [128, (w,d)]
                            diff = medp.tile([128, WWIN * HD], BF16, tag="diff",
                                             bufs=3)
                            diff_h[h] = diff
                            d3 = diff[:].rearrange("p (w d) -> p w d", w=8)
                            t0 = medp.tile([128, WWIN * HD], BF16, tag="t0")
                            nc.vector.tensor_tensor(
                                out=t0[:].rearrange("p (w a b) -> p w a b", w=8, a=2),
                                in0=usl[h].rearrange("p (a b) -> p a b", a=2)
                                    .unsqueeze(1).to_broadcast((128, 8, 2, 32)),
                                in1=cosb[:].rearrange("p (w f) -> p w f", w=8)
                                    .unsqueeze(2).to_broadcast((128, 8, 2, 32)),
                                op=ALU.mult)
                            t1 = medp.tile([128, WWIN * HD], BF16, tag="t0")
                            nc.vector.tensor_tensor(
                                out=t1[:].rearrange("p (w a b) -> p w a b", w=8, a=2),
                                in0=uRsl[h].rearrange("p (a b) -> p a b", a=2)
                                    .unsqueeze(1).to_broadcast((128, 8, 2, 32)),
                                in1=sinb[:].rearrange("p (w f) -> p w f", w=8)
                                    .unsqueeze(2).to_broadcast((128, 8, 2, 32)),
                                op=ALU.mult)
                            nc.vector.tensor_tensor(out=t0[:], in0=t0[:], in1=t1[:],
                                                    op=ALU.add)
                            nc.vector.tensor_tensor(
                                out=d3,
                                in0=t0[:].rearrange("p (w d) -> p w d", w=8),
                                in1=gbv[:, :, h * HD:(h + 1) * HD],
                                op=ALU.subtract)
                            # einsum1 products on GPSIMD (overlaps DVE)
                            prod = prodp.tile([128, WWIN * R * HD], BF16, tag="prodb")
                            prod_h[h] = prod
                            nc.gpsimd.tensor_tensor(
                                out=prod[:].rearrange("p (w r d) -> p w r d",
                                                      w=8, r=R),
                                in0=lam4,
                                in1=d3.unsqueeze(2).to_broadcast((128, 8, R, HD)),
                                op=ALU.mult)
                        ld2_h = {}
                        for h in pair:
                            # einsum1 reduce: Ld[w,r] = sum_d Lam*diff
                            ld = smlp.tile([128, WWIN * R], F32, tag="ld", bufs=3)
                            nc.vector.tensor_reduce(
                                ld[:].rearrange("p (w r) -> p w r", w=8),
                                prod_h[h][:].rearrange("p (w r d) -> p w r d",
                                                       w=8, r=R),
                                axis=AX.X, op=ALU.add)
                            ld2 = smlp.tile([128, WWIN * R], F32, tag="ld2", bufs=3)
                            ld2_h[h] = ld2
                            nc.vector.tensor_tensor(ld2[:], ld[:], rec_h[h][:],
                                                    op=ALU.mult)
                            # einsum2 products on GPSIMD: prod2 (w, d, r)
                            prod2 = prodp.tile([128, WWIN * HD * R], BF16,
                                               tag="prodb2")
                            prod_h[h] = prod2
                            lam4 = lam_h[h][:].rearrange(
                                "p (w r d) -> p w r d", w=8, r=R)
                            nc.gpsimd.tensor_tensor(
                                out=prod2[:].rearrange("p (w d r) -> p w d r",
                                                       w=8, d=HD),
                                in0=lam4.transpose([0, 1, 3, 2]),
                                in1=ld2[:].rearrange("p (w r) -> p w r", w=8)
                                    .unsqueeze(2).to_broadcast((128, 8, HD, R)),
                                op=ALU.mult)
                        for h in pair:
                            # einsum2 reduce over r, + alpha*diff, then sum over w
                            res = smlp.tile([128, WWIN * HD], F32, tag="res")
                            nc.vector.tensor_reduce(
                                res[:].rearrange("p (w d) -> p w d", w=8),
                                prod_h[h][:].rearrange("p (w d r) -> p w d r",
                                                       w=8, d=HD),
                                axis=AX.X, op=ALU.add)
                            adiff = smlp.tile([128, WWIN * HD], BF16, tag="adiff")
                            nc.vector.tensor_tensor(adiff[:], diff_h[h][:],
                                                    alpharep[:], op=ALU.mult)
                            nc.vector.tensor_tensor(res[:], res[:], adiff[:],
                                                    op=ALU.add)
                            rsum = smlp.tile([128, HD], F32, tag="rsum")
                            nc.vector.tensor_reduce(
                                rsum[:],
                                res[:].rearrange("p (w d) -> p d w", w=8),
                                axis=AX.X, op=ALU.add)
                            # u -= step*rsum ; uR via rotate_half identity
                            nc.vector.scalar_tensor_tensor(
                                out=usl[h], in0=rsum[:], scalar=stpn_sb[:, stc],
                                in1=usl[h], op0=ALU.mult, op1=ALU.add)
                            nc.vector.scalar_tensor_tensor(
                                out=uRsl[h][:, 0:32], in0=rsum[:, 32:64],
                                scalar=stp_sb[:, stc], in1=uRsl[h][:, 0:32],
                                op0=ALU.mult, op1=ALU.add)
                            nc.vector.scalar_tensor_tensor(
                                out=uRsl[h][:, 32:64], in0=rsum[:, 0:32],
                                scalar=stpn_sb[:, stc], in1=uRsl[h][:, 32:64],
                                op0=ALU.mult, op1=ALU.add)

            # ---------- output projection: y = u @ Wo + bo ----------
            Wo_t = load_w(Wo_d)
            for lt in range(NT):
                psy = psp.tile([128, 512], F32, space="PSUM", tag="mmps")
                nc.tensor.matmul(psy[:, :], ones1[:1, :128], bo_sb[:1, :],
                                 start=True, stop=False)
                for dc in range(4):
                    pst = psp.tile([128, 128], F32, space="PSUM", tag="small")
                    nc.tensor.transpose(
                        out=pst[:, :], in_=u_sb[lt][:, dc * 128:(dc + 1) * 128],
                        identity=ident)
                    uT = ldp.tile([128, 128], F32, tag="uT")
                    nc.scalar.copy(uT[:], pst[:, :])
                    nc.tensor.matmul(psy[:, :], uT[:], Wo_t[:, dc * D:(dc + 1) * D],
                                     start=False, stop=(dc == 3))
                ystg = ldp.tile([128, 512], F32, tag="stg")
                nc.scalar.copy(ystg[:], psy[:, :])
                nc.sync.dma_start(out=y_d[lt * 128:(lt + 1) * 128, :], in_=ystg[:])

    nc.finalize()
    return nc


def _rot_cols(Wm):
    """Fold rotate_half into output columns: out cols = [-cols(h, hi), cols(h, lo)]."""
    W4 = Wm.reshape(-1, H, 2, HD // 2)
    out = np.concatenate([-W4[:, :, 1], W4[:, :, 0]], axis=2)
    return np.ascontiguousarray(out.reshape(Wm.shape))


def make_in_maps(inputs):
    """Host-side prep: slice/transpose inputs into the 8 per-core input maps."""
    target = np.asarray(inputs["target"], np.float32)
    context = np.asarray(inputs["context"], np.float32)
    Wt = np.asarray(inputs["Wt"], np.float32)
    bt = np.asarray(inputs["bt"], np.float32)
    Wc = np.asarray(inputs["Wc"], np.float32)
    bc = np.asarray(inputs["bc"], np.float32)
    Ws1 = np.asarray(inputs["Ws1"], np.float32)
    bs1 = np.asarray(inputs["bs1"], np.float32)
    Ws2 = np.asarray(inputs["Ws2"], np.float32)
    Wa1 = np.asarray(inputs["Wa1"], np.float32)
    ba1 = np.asarray(inputs["ba1"], np.float32)
    Wa2 = np.asarray(inputs["Wa2"], np.float32)
    ba2 = np.asarray(inputs["ba2"], np.float32)
    Wl1 = np.asarray(inputs["Wl1"], np.float32)
    bl1 = np.asarray(inputs["bl1"], np.float32)
    Wl2 = np.asarray(inputs["Wl2"], np.float32)
    bl2 = np.asarray(inputs["bl2"], np.float32)
    step_sizes = np.asarray(inputs["step_sizes"], np.float32)
    Wo = np.asarray(inputs["Wo"], np.float32)
    bo = np.asarray(inputs["bo"], np.float32)


    import ml_dtypes
    Ws2bd = np.zeros((128, 8), np.float32)
    for ls in range(8):
        Ws2bd[ls * 16:(ls + 1) * 16, ls] = Ws2[:, 0]
    Wl2s = np.zeros((128, H * R * HD), np.float32)
    for s in range(4):
        Wl2s[32 * s:32 * s + EH, :] = Wl2
        Wl2s[32 * s + EH, :] = bl2
    Wl2s = Wl2s.astype(ml_dtypes.bfloat16)
    invf = (1.0 / (10000.0 ** (np.arange(0, HD, 2, dtype=np.float32) / HD)))[None, :]

    common = dict(
        Wt=Wt, WtR=_rot_cols(Wt), Wc=Wc, Wo=Wo,
        bt=bt[None, :], btR=_rot_cols(bt[None, :]), bc=bc[None, :], bo=bo[None, :],
        Wtr3=np.ascontiguousarray(np.concatenate([Ws1[:D], Wa1[:D], Wl1[:D]], axis=1)),
        Ws1c=np.ascontiguousarray(Ws1[D:]),
        Wacl=np.ascontiguousarray(np.concatenate([Wa1[D:], Wl1[D:]], axis=1)),
        bs1=bs1[None, :],
        bacl=np.concatenate([ba1, bl1])[None, :],
        Ws2bd=Ws2bd, Wa2=np.ascontiguousarray(Wa2.T),
        ba2=np.asarray(ba2, np.float32).reshape(1, 1), Wl2=Wl2s,
        invf=np.ascontiguousarray(invf, np.float32),
    )

    in_maps = []
    for c in range(8):
        b, rc = c // 4, c % 4
        rows = slice(rc * LC, (rc + 1) * LC)
        stp = np.ascontiguousarray(
            step_sizes[:, rows].reshape(T, NT, 128).transpose(2, 0, 1)
            .reshape(128, T * NT))
        lcol = np.ascontiguousarray(
            (rc * LC + np.arange(LC, dtype=np.float32)).reshape(NT, 128).T)
        m = dict(common)
        m.update(
            tT=np.ascontiguousarray(target[b, rows].T),
            cT=np.ascontiguousarray(context[b].T),
            stp=stp, lcol=lcol,
        )
        in_maps.append(m)
    return in_maps


_NC_CACHE = {}


def kernel(**inputs):
    if "nc" not in _NC_CACHE:
        _NC_CACHE["nc"] = build_program()
    nc = _NC_CACHE["nc"]
    in_maps = make_in_maps(inputs)
    res = run_bass_kernel_spmd(nc, in_maps, list(range(8)))
    out = np.empty((B, L, D), np.float32)
    for c in range(8):
        b, rc = c // 4, c % 4
        out[b, rc * LC:(rc + 1) * LC] = res.results[c]["y"]
    return out


# revision 12
# speedup vs baseline: 1.2054x; 1.0495x over previous
"""CrossConsensus kernel for 8 Trainium2 NeuronCores.

Sharding: data-parallel over B*L rows. Core c handles batch b=c//4,
target rows [ (c%4)*512, (c%4+1)*512 ).  All computation is row-local
(edge_i = repeat(arange(L), 8) means each edge scatters back to its own
source row), so there are no collectives; each core needs its target
row-chunk plus the full context of its batch.

v3: r8 einsums (alpha via per-l-tile alpharep), einsum products on
GPSIMD, fused across-w norms, ACT-engine PSUM evacuations, no uR state
(rotate_half via a reversed-stride view of u and sign-folded sin), and
4-head pipeline groups so DVE keeps working while GPSIMD computes.
"""

import math

import numpy as np

import concourse.bass as bass
import concourse.bacc as bacc
import concourse.tile as tile
from concourse import mybir
from concourse.bass_utils import run_bass_kernel_spmd
from concourse.masks import make_identity

F32 = mybir.dt.float32
BF16 = mybir.dt.bfloat16
U32 = mybir.dt.uint32
AX = mybir.AxisListType
ALU = mybir.AluOpType
ACTF = mybir.ActivationFunctionType

# problem constants (hardcoded per the harness contract)
B, L, K, D = 2, 2048, 2048, 512
H, R, WWIN, T, EH = 8, 8, 8, 2, 16
HD = D // H            # 64
LC = L * B // 8        # 512 rows per core
NT = LC // 128         # 4 l-tiles per core
KT = K // 128          # 16 k-tiles
CROW = D + 2 * EH      # 544: gather-table row [v(512) | ca(16) | cl(16)]
TWO_PI = 2.0 * math.pi
HG = 4                 # heads per pipeline group


def build_program():
    nc = bacc.Bacc()

    # ---------------- external I/O ----------------
    tT = nc.dram_tensor("tT", [D, LC], F32, kind="ExternalInput")        # target^T
    cT = nc.dram_tensor("cT", [D, K], F32, kind="ExternalInput")         # context^T
    Wt_d = nc.dram_tensor("Wt", [D, D], F32, kind="ExternalInput")
    Wc_d = nc.dram_tensor("Wc", [D, D], F32, kind="ExternalInput")
    Wo_d = nc.dram_tensor("Wo", [D, D], F32, kind="ExternalInput")
    bpack_d = nc.dram_tensor("bpack", [128, D], F32, kind="ExternalInput")  # bt@0|bc@32|bo@64
    Wtr3_d = nc.dram_tensor("Wtr3", [D, 48], F32, kind="ExternalInput")  # [Ws1t|Wa1t|Wl1t]
    Ws1c_d = nc.dram_tensor("Ws1c", [D, EH], F32, kind="ExternalInput")
    Wacl_d = nc.dram_tensor("Wacl", [D, 32], F32, kind="ExternalInput")  # [Wa1c|Wl1c]
    bs1_d = nc.dram_tensor("bs1", [1, EH], F32, kind="ExternalInput")
    bacl_d = nc.dram_tensor("bacl", [1, 32], F32, kind="ExternalInput")  # [ba1|bl1]
    Ws2bd_d = nc.dram_tensor("Ws2bd", [128, 8], F32, kind="ExternalInput")
    Wa2_d = nc.dram_tensor("Wa2", [1, EH], F32, kind="ExternalInput")
    ba2_d = nc.dram_tensor("ba2", [1, 1], F32, kind="ExternalInput")
    Wl2_d = nc.dram_tensor("Wl2", [128, H * R * HD], BF16, kind="ExternalInput")
    stp_d = nc.dram_tensor("stp", [128, T * NT], F32, kind="ExternalInput")
    lcol_d = nc.dram_tensor("lcol", [128, NT], F32, kind="ExternalInput")
    invf_d = nc.dram_tensor("invf", [1, HD // 2], F32, kind="ExternalInput")
    y_d = nc.dram_tensor("y", [LC, D], F32, kind="ExternalOutput")

    # internal DRAM gather table
    Tctx = nc.dram_tensor("Tctx", [K, CROW], BF16)

    # ---------------- persistent SBUF (static allocs, before pools) ----------
    ident = nc.alloc_sbuf_tensor("ident", [128, 128], F32).ap()
    ones1 = nc.alloc_sbuf_tensor("ones1", [128, 512], F32).ap()
    u_sb = [nc.alloc_sbuf_tensor(f"u{i}", [128, D], F32).ap() for i in range(NT)]
    trio = [nc.alloc_sbuf_tensor(f"trio{i}", [128, 48], F32).ap() for i in range(NT)]
    Wl2_sb = nc.alloc_sbuf_tensor("Wl2sb", [128, H * R * HD], BF16).ap()
    cpTrep = nc.alloc_sbuf_tensor("cpTrep", [128, K], F32).ap()
    tpbT = nc.alloc_sbuf_tensor("tpbT", [128, NT * 16], F32).ap()
    invf_sb = nc.alloc_sbuf_tensor("invfsb", [128, HD // 2], F32).ap()
    wa2_sb = nc.alloc_sbuf_tensor("wa2sb", [128, EH], F32).ap()
    ba2_sb = nc.alloc_sbuf_tensor("ba2sb", [128, 1], F32).ap()
    stp_sb = nc.alloc_sbuf_tensor("stpsb", [128, T * NT], F32).ap()
    stpn_sb = nc.alloc_sbuf_tensor("stpnsb", [128, T * NT], F32).ap()
    lcol_sb = nc.alloc_sbuf_tensor("lcolsb", [128, NT], F32).ap()
    bs1_sb = nc.alloc_sbuf_tensor("bs1sb", [1, EH], F32).ap()
    bacl_sb = nc.alloc_sbuf_tensor("baclsb", [1, 32], F32).ap()
    bpack_sb = nc.alloc_sbuf_tensor("bpacksb", [128, D], F32).ap()
    Ws2bd_sb = nc.alloc_sbuf_tensor("ws2bdsb", [128, 8], F32).ap()
    Wtr3_sb = nc.alloc_sbuf_tensor("wtr3sb", [128, 4 * 48], F32).ap()
    Wacl_sb = nc.alloc_sbuf_tensor("waclsb", [128, 4 * 32], F32).ap()
    halfpi = nc.alloc_sbuf_tensor("halfpi", [128, 1], F32).ap()
    onec = nc.alloc_sbuf_tensor("onec", [128, 1], F32).ap()
    onesb = nc.alloc_sbuf_tensor("onesb", [1, 256], BF16).ap()

    with tile.TileContext(nc) as tc:
        with (
            tc.tile_pool(name="ld", bufs=2) as ldp,             # small staging tiles
            tc.tile_pool(name="gbp", bufs=2) as gbp,            # gather block
            tc.tile_pool(name="lamp", bufs=HG) as lamp,         # Lam (one per group head)
            tc.tile_pool(name="prodp", bufs=2) as prodp,        # einsum products
            tc.tile_pool(name="med", bufs=2) as medp,
            tc.tile_pool(name="sml", bufs=2) as smlp,
            tc.tile_pool(name="wp", bufs=1) as wp,
            tc.tile_pool(name="ps", bufs=2, space="PSUM") as psp,
            tc.tile_pool(name="ps4", bufs=4, space="PSUM") as ps4p,
        ):
            # ---------- constants ----------
            make_identity(nc, ident)
            nc.vector.memset(ones1, 1.0)
            nc.vector.memset(halfpi, math.pi / 2)
            nc.vector.memset(onec, 1.0)
            nc.vector.memset(onesb, 1.0)
            nc.sync.dma_start(out=invf_sb, in_=invf_d[:].partition_broadcast(128))
            nc.sync.dma_start(out=wa2_sb, in_=Wa2_d[:].partition_broadcast(128))
            nc.sync.dma_start(out=ba2_sb, in_=ba2_d[:].partition_broadcast(128))
            nc.sync.dma_start(out=lcol_sb, in_=lcol_d[:])
            nc.sync.dma_start(out=bs1_sb, in_=bs1_d[:])
            nc.sync.dma_start(out=bacl_sb, in_=bacl_d[:])
            nc.sync.dma_start(out=bpack_sb, in_=bpack_d[:])
            nc.sync.dma_start(out=Ws2bd_sb, in_=Ws2bd_d[:])
            nc.sync.dma_start(out=Wl2_sb, in_=Wl2_d[:])
            for dc in range(4):
                sl = slice(dc * 128, (dc + 1) * 128)
                nc.sync.dma_start(out=Wtr3_sb[:, dc * 48:(dc + 1) * 48], in_=Wtr3_d[sl, :])
                nc.sync.dma_start(out=Wacl_sb[:, dc * 32:(dc + 1) * 32], in_=Wacl_d[sl, :])

            bt_b = bpack_sb[0:1, :]
            bc_b = bpack_sb[32:33, :]
            bo_b = bpack_sb[64:65, :]

            def load_w(dram):
                t = wp.tile([128, 4 * D], F32, tag="wrhs")
                for dc in range(4):
                    nc.sync.dma_start(out=t[:, dc * D:(dc + 1) * D],
                                      in_=dram[dc * 128:(dc + 1) * 128, :])
                return t

            def softplus(dst, src, bias_ap, tmp_pool, tmp_tag):
                """dst = softplus(src + bias) = relu(x) + ln(1+exp(-|x|)).
                No softplus HW table; composed from abs/exp/ln (one table set)."""
                shp = [src.shape[0], src.free_size()]
                a = tmp_pool.tile(shp, F32, tag=tmp_tag)
                if bias_ap is None:
                    nc.scalar.activation(a[:], src, ACTF.Abs)
                    nc.vector.tensor_scalar(dst, src, 0.0, scalar2=None, op0=ALU.max)
                else:
                    nc.scalar.activation(a[:], src, ACTF.Abs, bias=bias_ap)
                    nc.vector.tensor_scalar(dst, src, bias_ap, scalar2=0.0,
                                            op0=ALU.add, op1=ALU.max)
                nc.scalar.activation(a[:], a[:], ACTF.Exp, scale=-1.0)
                nc.scalar.activation(a[:], a[:], ACTF.Ln, bias=onec[:, 0:1])
                nc.vector.tensor_tensor(dst, dst, a[:], op=ALU.add)

            stp_raw = smlp.tile([128, T * NT], F32, tag="stpraw")
            nc.sync.dma_start(out=stp_raw[:], in_=stp_d[:])
            softplus(stp_sb, stp_raw[:], None, smlp, "sptmp")
            nc.vector.tensor_scalar_mul(stpn_sb, stp_sb, -1.0)

            # ---------- dense projections ----------
            def mm_rows(out_ap, lhsT_dram, tix, w_sb, ncol, bias_sb, evac="v"):
                """out[128 rows of tile tix, ncol] = lhsT_dram[:, tile].T @ W (+ bias)."""
                ps = psp.tile([128, 512], F32, space="PSUM", tag="mmps")
                have_bias = bias_sb is not None
                if have_bias:
                    bb = bias_sb.base_partition()
                    nc.tensor.matmul(ps[:, :ncol], ones1[bb:bb + 1, :128],
                                     bias_sb[:1, :ncol], start=True, stop=False)
                for dc in range(4):
                    lh = ldp.tile([128, 128], F32, tag="lhst")
                    nc.sync.dma_start(
                        out=lh[:], in_=lhsT_dram[dc * 128:(dc + 1) * 128,
                                                 tix * 128:(tix + 1) * 128])
                    nc.tensor.matmul(ps[:, :ncol], lh[:],
                                     w_sb[:, dc * ncol:(dc + 1) * ncol],
                                     start=(not have_bias and dc == 0),
                                     stop=(dc == 3))
                if evac == "v":
                    nc.scalar.copy(out_ap, ps[:, :ncol])
                else:  # DRAM destination: stage through SBUF (DMA can't read PSUM)
                    stg = ldp.tile([128, 512], BF16, tag="stgb")
                    nc.scalar.copy(stg[:, :ncol], ps[:, :ncol])
                    nc.sync.dma_start(out=out_ap, in_=stg[:, :ncol])

            Wt_t = load_w(Wt_d)
            for lt in range(NT):
                mm_rows(u_sb[lt][:], tT, lt, Wt_t[:], D, bt_b)
            for lt in range(NT):
                mm_rows(trio[lt][:], tT, lt, Wtr3_sb, 48, None)

            Wc_t = load_w(Wc_d)
            for kt in range(KT):
                mm_rows(Tctx[kt * 128:(kt + 1) * 128, 0:D], cT, kt, Wc_t[:], D,
                        bc_b, evac="dma")
                mm_rows(Tctx[kt * 128:(kt + 1) * 128, D:D + 32], cT, kt, Wacl_sb, 32,
                        bacl_sb, evac="dma")

            # tpbT: per-octet score bias columns, partition layout p = ls*16 + e
            for lt in range(NT):
                for oc in range(16):
                    nc.sync.dma_start(
                        out=tpbT[:, lt * 16 + oc:lt * 16 + oc + 1],
                        in_=trio[lt][oc * 8:(oc + 1) * 8, 0:EH])

            # cpT [16, K] = Ws1c.T @ context^T + bs1, then replicate 8x on partitions
            cpT_t = medp.tile([EH, K], F32, tag="cpTt", bufs=1)
            cpT = cpT_t[:]
            for nt4 in range(4):
                nsl = slice(nt4 * 512, (nt4 + 1) * 512)
                ps = psp.tile([128, 512], F32, space="PSUM", tag="mmps")
                nc.tensor.matmul(ps[:EH, :], bs1_sb[:1, :], ones1[:1, :512],
                                 start=True, stop=False)
                for dc in range(4):
                    lh = ldp.tile([128, EH], F32, tag="lhst16")
                    nc.sync.dma_start(out=lh[:],
                                      in_=Ws1c_d[dc * 128:(dc + 1) * 128, :])
                    rh = ldp.tile([128, 512], F32, tag="ctchunk")
                    nc.sync.dma_start(out=rh[:], in_=cT[dc * 128:(dc + 1) * 128, nsl])
                    nc.tensor.matmul(ps[:EH, :], lh[:], rh[:],
                                     start=False, stop=(dc == 3))
                nc.vector.tensor_copy(cpT[:, nsl], ps[:EH, :])
            for ls in range(8):
                nc.sync.dma_start(out=cpTrep[ls * 16:(ls + 1) * 16, :], in_=cpT[:, :])

            # ---------- per l-tile ----------
            for lt in range(NT):
                # ----- scores + top-8 -----
                scores = medp.tile([128, K], F32, tag="scores", bufs=1)
                for oc in range(16):
                    for hf in range(2):
                        g_sc = medp.tile([128, K // 2], F32, tag="gsc")
                        nc.scalar.activation(
                            g_sc[:], cpTrep[:, hf * 1024:(hf + 1) * 1024], ACTF.Gelu,
                            bias=tpbT[:, lt * 16 + oc:lt * 16 + oc + 1])
                        for nq in range(2):
                            col = hf * 1024 + nq * 512
                            pssc = psp.tile([8, 512], F32, space="PSUM", tag="small")
                            nc.tensor.matmul(pssc[:, :], Ws2bd_sb[:],
                                             g_sc[:, nq * 512:(nq + 1) * 512],
                                             start=True, stop=True)
                            sstg = medp.tile([8, 512], F32, tag="sstg")
                            nc.scalar.copy(sstg[:], pssc[:, :])
                            nc.sync.dma_start(
                                out=scores[oc * 8:(oc + 1) * 8, col:col + 512],
                                in_=sstg[:])
                mx8 = smlp.tile([128, 8], F32, tag="mx8")
                idx = smlp.tile([128, 8], U32, tag="idx")
                nc.vector.max(out=mx8[:], in_=scores[:])
                nc.vector.max_index(out=idx[:], in_max=mx8[:], in_values=scores[:])

                # ----- gather context-side rows -----
                gb = gbp.tile([128, WWIN * CROW], BF16, tag="gb")
                gbv = gb[:].rearrange("p (w c) -> p w c", w=8)
                for w in range(WWIN):
                    nc.gpsimd.indirect_dma_start(
                        out=gb[:, w * CROW:(w + 1) * CROW],
                        out_offset=None,
                        in_=Tctx[:, :],
                        in_offset=bass.IndirectOffsetOnAxis(ap=idx[:, w:w + 1], axis=0),
                    )

                # ----- per-edge angles -----
                jf = smlp.tile([128, 8], F32, tag="jf")
                nc.vector.tensor_copy(jf[:], idx[:])
                delta = smlp.tile([128, 8], F32, tag="delta")
                nc.vector.tensor_scalar(delta[:], jf[:], lcol_sb[:, lt:lt + 1],
                                        scalar2=None, op0=ALU.subtract)
                ang = medp.tile([128, 8 * 32], F32, tag="ang")
                nc.vector.tensor_tensor(
                    out=ang[:].rearrange("p (w f) -> p w f", w=8),
                    in0=delta[:].unsqueeze(2).to_broadcast((128, 8, 32)),
                    in1=invf_sb[:].unsqueeze(1).to_broadcast((128, 8, 32)),
                    op=ALU.mult)
                # range-reduce to [-pi, pi]: x - 2pi*round(x/2pi), round via
                # the +/- 1.5*2^23 magic-number trick (no mod/floor on DVE ISA)
                MAGIC = 1.5 * 2.0 ** 23
                angt = medp.tile([128, 8 * 32], F32, tag="angt")
                nc.vector.tensor_scalar_mul(angt[:], ang[:], 1.0 / TWO_PI)
                angr = medp.tile([128, 8 * 32], F32, tag="angr")
                nc.vector.tensor_scalar(angr[:], angt[:], MAGIC, scalar2=MAGIC,
                                        op0=ALU.add, op1=ALU.subtract)
                nc.vector.tensor_sub(angt[:], angt[:], angr[:])
                nc.vector.tensor_scalar_mul(ang[:], angt[:], TWO_PI)
                cosb = medp.tile([128, 8 * 32], F32, tag="cosb")
                sinb = medp.tile([128, 8 * 32], F32, tag="sinb")
                nc.scalar.activation(sinb[:], ang[:], ACTF.Sin, scale=-1.0)
                nc.vector.tensor_scalar_mul(angr[:], ang[:], -1.0)
                nc.vector.tensor_max(angt[:], ang[:], angr[:])
                nc.scalar.activation(cosb[:], angt[:], ACTF.Sin, scale=-1.0,
                                     bias=halfpi[:, 0:1])
                # sinb2[w, a, f]: a=0 -> -sin (pairs with -u_hi), a=1 -> +sin
                sinb2 = medp.tile([128, 8 * 2 * 32], F32, tag="sinb2")
                s2v = sinb2[:].rearrange("p (w a f) -> p w a f", w=8, a=2)
                nc.vector.tensor_scalar_mul(
                    s2v[:, :, 0, :], sinb[:].rearrange("p (w f) -> p w f", w=8),
                    -1.0)
                nc.vector.tensor_copy(
                    s2v[:, :, 1, :], sinb[:].rearrange("p (w f) -> p w f", w=8))

                # ----- alphas = softplus(gelu(ta+ca) @ Wa2 + ba2) -----
                ha = smlp.tile([128, 8 * EH], F32, tag="ha")
                nc.vector.tensor_tensor(
                    out=ha[:].rearrange("p (w c) -> p w c", w=8),
                    in0=trio[lt][:, 16:32].unsqueeze(1).to_broadcast((128, 8, EH)),
                    in1=gbv[:, :, D:D + EH],
                    op=ALU.add)
                nc.scalar.activation(ha[:], ha[:], ACTF.Gelu)
                haw = smlp.tile([128, 8 * EH], F32, tag="haw")
                nc.vector.tensor_tensor(
                    out=haw[:].rearrange("p (w c) -> p w c", w=8),
                    in0=ha[:].rearrange("p (w c) -> p w c", w=8),
                    in1=wa2_sb[:].unsqueeze(1).to_broadcast((128, 8, EH)),
                    op=ALU.mult)
                alphas = smlp.tile([128, 8], F32, tag="alphas")
                nc.vector.tensor_reduce(alphas[:], haw[:].rearrange(
                    "p (w c) -> p w c", w=8), axis=AX.X, op=ALU.add)
                softplus(alphas[:], alphas[:], ba2_sb[:, 0:1], smlp, "sptmp")
                # alpharep [128, (w d)] = alphas replicated over d (t/h-invariant)
                alpharep = smlp.tile([128, WWIN * HD], F32, tag="alpharep")
                nc.vector.tensor_copy(
                    alpharep[:].rearrange("p (w d) -> p w d", w=8),
                    alphas[:].unsqueeze(2).to_broadcast((128, 8, HD)))

                # ----- g = gelu(tl + cl) and per-w transposes -----
                gmat = smlp.tile([128, 8 * EH], F32, tag="gmat")
                nc.vector.tensor_tensor(
                    out=gmat[:].rearrange("p (w c) -> p w c", w=8),
                    in0=trio[lt][:, 32:48].unsqueeze(1).to_broadcast((128, 8, EH)),
                    in1=gbv[:, :, D + EH:D + 2 * EH],
                    op=ALU.add)
                nc.scalar.activation(gmat[:], gmat[:], ACTF.Gelu)
                gT4 = gbp.tile([128, 2 * 128], BF16, tag="gT4")  # 2 quads side by side
                nc.vector.memset(gT4[:], 0.0)
                for s4 in range(4):  # bias row (constant 1) for the bl2 fold
                    nc.sync.dma_start(out=gT4[32 * s4 + EH:32 * s4 + EH + 1, :],
                                      in_=onesb[:1, 0:256])
                for w in range(WWIN):
                    q, s = w // 4, w % 4
                    pst = psp.tile([EH, 128], F32, space="PSUM", tag="small")
                    nc.tensor.transpose(
                        out=pst[:, :],
                        in_=gmat[:].rearrange("p (w c) -> p w c", w=8)[:, w, :],
                        identity=ident)
                    nc.vector.tensor_copy(
                        gT4[32 * s:32 * s + EH, q * 128:(q + 1) * 128], pst[:, :])

                # ----- 4-head pipeline groups -----
                for hg in range(H // HG):
                    heads = range(hg * HG, (hg + 1) * HG)
                    lam_h = {}
                    rec_h = {}
                    for h in heads:
                        # Lam layout: (w, r, d) bf16, contiguous
                        Lam = lamp.tile([128, WWIN * R * HD], BF16, tag="lam")
                        lam_h[h] = Lam
                        for w in range(WWIN):
                            q, s = w // 4, w % 4
                            psl = ps4p.tile([128, 512], F32, space="PSUM", tag="lamps")
                            nc.tensor.matmul(
                                psl[:, :], gT4[32 * s:32 * s + 32, q * 128:(q + 1) * 128],
                                Wl2_sb[32 * s:32 * s + 32, h * R * HD:(h + 1) * R * HD],
                                start=True, stop=True, tile_position=(32 * s, 0))
                            nc.scalar.copy(
                                Lam[:, w * R * HD:(w + 1) * R * HD], psl[:, :])
                        # squared row norms, fused across w: sq = Lam^2 (ACT)
                        sq = prodp.tile([128, WWIN * R * HD], BF16, tag="prodb")
                        nc.scalar.activation(sq[:], Lam[:], ACTF.Square)
                        n2 = smlp.tile([128, WWIN * R], F32, tag="n2")
                        nc.vector.tensor_reduce(
                            n2[:].rearrange("p (w r) -> p w r", w=8),
                            sq[:].rearrange("p (w r d) -> p w r d", w=8, r=R),
                            axis=AX.X, op=ALU.add)
                        nrm = smlp.tile([128, WWIN * R], F32, tag="nrm")
                        nc.vector.tensor_scalar_max(nrm[:], n2[:], 1e-24)
                        rec = smlp.tile([128, WWIN * R], F32, tag="rec", bufs=HG)
                        rec_h[h] = rec
                        nc.vector.reciprocal(rec[:], nrm[:])

                    usl = {h: u_sb[lt][:, h * HD:(h + 1) * HD] for h in heads}

                    def emit_diff_e1p(h, t):
                        lam4 = lam_h[h][:].rearrange(
                            "p (w r d) -> p w r d", w=8, r=R)
                        # diff = u_i*cos + rot_half(u_i)*sin - v_j   [128, (w,d)]
                        diff = medp.tile([128, WWIN * HD], BF16, tag="diff",
                                         bufs=HG + 1)
                        d3 = diff[:].rearrange("p (w d) -> p w d", w=8)
                        t0 = medp.tile([128, WWIN * HD], BF16, tag="t0", bufs=3)
                        nc.vector.tensor_tensor(
                            out=t0[:].rearrange("p (w a b) -> p w a b", w=8, a=2),
                            in0=usl[h].rearrange("p (a b) -> p a b", a=2)
                                .unsqueeze(1).to_broadcast((128, 8, 2, 32)),
                            in1=cosb[:].rearrange("p (w f) -> p w f", w=8)
                                .unsqueeze(2).to_broadcast((128, 8, 2, 32)),
                            op=ALU.mult)
                        t1 = medp.tile([128, WWIN * HD], BF16, tag="t0", bufs=3)
                        nc.vector.tensor_tensor(
                            out=t1[:].rearrange("p (w a b) -> p w a b", w=8, a=2),
                            in0=usl[h].rearrange("p (a b) -> p a b", a=2)[:, ::-1, :]
                                .unsqueeze(1).to_broadcast((128, 8, 2, 32)),
                            in1=s2v,
                            op=ALU.mult)
                        nc.vector.tensor_tensor(out=t0[:], in0=t0[:], in1=t1[:],
                                                op=ALU.add)
                        nc.vector.tensor_tensor(
                            out=d3,
                            in0=t0[:].rearrange("p (w d) -> p w d", w=8),
                            in1=gbv[:, :, h * HD:(h + 1) * HD],
                            op=ALU.subtract)
                        # einsum1 products on GPSIMD (overlaps DVE)
                        prod = prodp.tile([128, WWIN * R * HD], BF16, tag="prodb",
                                          bufs=2)
                        nc.gpsimd.tensor_tensor(
                            out=prod[:].rearrange("p (w r d) -> p w r d", w=8, r=R),
                            in0=lam4,
                            in1=d3.unsqueeze(2).to_broadcast((128, 8, R, HD)),
                            op=ALU.mult)
                        return diff, prod

                    def emit_e1r_e2p(h, t, prod):
                        lam4 = lam_h[h][:].rearrange(
                            "p (w r d) -> p w r d", w=8, r=R)
                        # einsum1 reduce: Ld[w,r] = sum_d Lam*diff
                        ld = smlp.tile([128, WWIN * R], F32, tag="ld", bufs=3)
                        nc.vector.tensor_reduce(
                            ld[:].rearrange("p (w r) -> p w r", w=8),
                            prod[:].rearrange("p (w r d) -> p w r d", w=8, r=R),
                            axis=AX.X, op=ALU.add)
                        ld2 = smlp.tile([128, WWIN * R], F32, tag="ld2", bufs=3)
                        nc.vector.tensor_tensor(ld2[:], ld[:], rec_h[h][:],
                                                op=ALU.mult)
                        # einsum2 products on GPSIMD: prod2 (w, d, r)
                        prod2 = prodp.tile([128, WWIN * HD * R], BF16, tag="prodb2",
                                           bufs=2)
                        nc.gpsimd.tensor_tensor(
                            out=prod2[:].rearrange("p (w d r) -> p w d r",
                                                   w=8, d=HD),
                            in0=lam4.transpose([0, 1, 3, 2]),
                            in1=ld2[:].rearrange("p (w r) -> p w r", w=8)
                                .unsqueeze(2).to_broadcast((128, 8, HD, R)),
                            op=ALU.mult)
                        return prod2

                    def emit_adiff(h, t, diff):
                        adiff = smlp.tile([128, WWIN * HD], BF16, tag="adiff",
                                          bufs=3)
                        nc.vector.tensor_tensor(adiff[:], diff[:], alpharep[:],
                                                op=ALU.mult)
                        return adiff

                    def emit_tail(h, t, prod2, adiff):
                        stc = slice(t * NT + lt, t * NT + lt + 1)
                        # einsum2 reduce over r, + alpha*diff, then sum over w
                        res = smlp.tile([128, WWIN * HD], F32, tag="res")
                        nc.vector.tensor_reduce(
                            res[:].rearrange("p (w d) -> p w d", w=8),
                            prod2[:].rearrange("p (w d r) -> p w d r",
                                               w=8, d=HD),
                            axis=AX.X, op=ALU.add)
                        nc.vector.tensor_tensor(res[:], res[:], adiff[:],
                                                op=ALU.add)
                        rsum = smlp.tile([128, HD], F32, tag="rsum")
                        nc.vector.tensor_reduce(
                            rsum[:],
                            res[:].rearrange("p (w d) -> p d w", w=8),
                            axis=AX.X, op=ALU.add)
                        # u -= step*rsum
                        nc.vector.scalar_tensor_tensor(
                            out=usl[h], in0=rsum[:], scalar=stpn_sb[:, stc],
                            in1=usl[h], op0=ALU.mult, op1=ALU.add)

                    # software-pipelined 2-iteration t-loop
                    st = {}
                    for h in heads:
                        st[h] = emit_diff_e1p(h, 0)
                    prod2s = {}
                    for h in heads:
                        prod2s[h] = emit_e1r_e2p(h, 0, st[h][1])
                    adiffs = {}
                    for h in heads:
                        adiffs[h] = emit_adiff(h, 0, st[h][0])
                    for h in heads:
                        emit_tail(h, 0, prod2s[h], adiffs[h])
                        st[h] = emit_diff_e1p(h, 1)
                    for h in heads:
                        prod2s[h] = emit_e1r_e2p(h, 1, st[h][1])
                    for h in heads:
                        adiffs[h] = emit_adiff(h, 1, st[h][0])
                    for h in heads:
                        emit_tail(h, 1, prod2s[h], adiffs[h])

            # ---------- output projection: y = u @ Wo + bo ----------
            Wo_t = load_w(Wo_d)
            for lt in range(NT):
                psy = psp.tile([128, 512], F32, space="PSUM", tag="mmps")
                nc.tensor.matmul(psy[:, :], ones1[64:65, :128], bo_b[:1, :],
                                 start=True, stop=False)
                for dc in range(4):
                    pst = psp.tile([128, 128], F32, space="PSUM", tag="small")
                    nc.tensor.transpose(
                        out=pst[:, :], in_=u_sb[lt][:, dc * 128:(dc + 1) * 128],
                        identity=ident)
                    uT = ldp.tile([128, 128], F32, tag="uT")
                    nc.scalar.copy(uT[:], pst[:, :])
                    nc.tensor.matmul(psy[:, :], uT[:], Wo_t[:, dc * D:(dc + 1) * D],
                                     start=False, stop=(dc == 3))
                ystg = ldp.tile([128, 512], F32, tag="stg")
                nc.scalar.copy(ystg[:], psy[:, :])
                nc.sync.dma_start(out=y_d[lt * 128:(lt + 1) * 128, :], in_=ystg[:])

    nc.finalize()
    return nc


def make_in_maps(inputs):
    """Host-side prep: slice/transpose inputs into the 8 per-core input maps."""
    target = np.asarray(inputs["target"], np.float32)
    context = np.asarray(inputs["context"], np.float32)
    Wt = np.asarray(inputs["Wt"], np.float32)
    bt = np.asarray(inputs["bt"], np.float32)
    Wc = np.asarray(inputs["Wc"], np.float32)
    bc = np.asarray(inputs["bc"], np.float32)
    Ws1 = np.asarray(inputs["Ws1"], np.float32)
    bs1 = np.asarray(inputs["bs1"], np.float32)
    Ws2 = np.asarray(inputs["Ws2"], np.float32)
    Wa1 = np.asarray(inputs["Wa1"], np.float32)
    ba1 = np.asarray(inputs["ba1"], np.float32)
    Wa2 = np.asarray(inputs["Wa2"], np.float32)
    ba2 = np.asarray(inputs["ba2"], np.float32)
    Wl1 = np.asarray(inputs["Wl1"], np.float32)
    bl1 = np.asarray(inputs["bl1"], np.float32)
    Wl2 = np.asarray(inputs["Wl2"], np.float32)
    bl2 = np.asarray(inputs["bl2"], np.float32)
    step_sizes = np.asarray(inputs["step_sizes"], np.float32)
    Wo = np.asarray(inputs["Wo"], np.float32)
    bo = np.asarray(inputs["bo"], np.float32)

    import ml_dtypes
    Ws2bd = np.zeros((128, 8), np.float32)  # cast to bf16 below
    for ls in range(8):
        Ws2bd[ls * 16:(ls + 1) * 16, ls] = Ws2[:, 0]
    Wl2s = np.zeros((128, H * R * HD), np.float32)
    for s in range(4):
        Wl2s[32 * s:32 * s + EH, :] = Wl2
        Wl2s[32 * s + EH, :] = bl2
    Wl2s = Wl2s.astype(ml_dtypes.bfloat16)
    invf = (1.0 / (10000.0 ** (np.arange(0, HD, 2, dtype=np.float32) / HD)))[None, :]
    bpack = np.zeros((128, D), np.float32)
    bpack[0] = bt
    bpack[32] = bc
    bpack[64] = bo

    common = dict(
        Wt=Wt, Wc=Wc, Wo=Wo, bpack=bpack,
        Wtr3=np.ascontiguousarray(np.concatenate([Ws1[:D], Wa1[:D], Wl1[:D]], axis=1)),
        Ws1c=np.ascontiguousarray(Ws1[D:]),
        Wacl=np.ascontiguousarray(np.concatenate([Wa1[D:], Wl1[D:]], axis=1)),
        bs1=bs1[None, :],
        bacl=np.concatenate([ba1, bl1])[None, :],
        Ws2bd=Ws2bd, Wa2=np.ascontiguousarray(Wa2.T),
        ba2=np.asarray(ba2, np.float32).reshape(1, 1), Wl2=Wl2s,
        invf=np.ascontiguousarray(invf, np.float32),
    )

    in_maps = []
    for c in range(8):
        b, rc = c // 4, c % 4
        rows = slice(rc * LC, (rc + 1) * LC)
        stp = np.ascontiguousarray(
            step_sizes[:, rows].reshape(T, NT, 128).transpose(2, 0, 1)
            .reshape(128, T * NT))
        lcol = np.ascontiguousarray(
            (rc * LC + np.arange(LC, dtype=np.float32)).reshape(NT, 128).T)
        m = dict(common)
        m.update(
            tT=np.ascontiguousarray(target[b, rows].T),
            cT=np.ascontiguousarray(context[b].T),
            stp=stp, lcol=lcol,
        )
        in_maps.append(m)
    return in_maps


_NC_CACHE = {}


def kernel(**inputs):
    if "nc" not in _NC_CACHE:
        _NC_CACHE["nc"] = build_program()
    nc = _NC_CACHE["nc"]
    in_maps = make_in_maps(inputs)
    res = run_bass_kernel_spmd(nc, in_maps, list(range(8)))
    out = np.empty((B, L, D), np.float32)
    for c in range(8):
        b, rc = c // 4, c % 4
        out[b, rc * LC:(rc + 1) * LC] = res.results[c]["y"]
    return out


# revision 16
# speedup vs baseline: 1.3779x; 1.1432x over previous
"""CrossConsensus kernel for 8 Trainium2 NeuronCores.

Sharding: data-parallel over B*L rows. Core c handles batch b=c//4,
target rows [ (c%4)*512, (c%4+1)*512 ).  All computation is row-local
(edge_i = repeat(arange(L), 8) means each edge scatters back to its own
source row), so there are no collectives; each core needs its target
row-chunk plus the full context of its batch.

v3: r8 einsums (alpha via per-l-tile alpharep), einsum products on
GPSIMD, fused across-w norms, ACT-engine PSUM evacuations, no uR state
(rotate_half via a reversed-stride view of u and sign-folded sin), and
4-head pipeline groups so DVE keeps working while GPSIMD computes.
"""

import math

import numpy as np

import concourse.bass as bass
import concourse.bacc as bacc
import concourse.tile as tile
from concourse import mybir
from concourse.bass_utils import run_bass_kernel_spmd
from concourse.masks import make_identity

F32 = mybir.dt.float32
BF16 = mybir.dt.bfloat16
U32 = mybir.dt.uint32
AX = mybir.AxisListType
ALU = mybir.AluOpType
ACTF = mybir.ActivationFunctionType

# problem constants (hardcoded per the harness contract)
B, L, K, D = 2, 2048, 2048, 512
H, R, WWIN, T, EH = 8, 8, 8, 2, 16
HD = D // H            # 64
LC = L * B // 8        # 512 rows per core
NT = LC // 128         # 4 l-tiles per core
KT = K // 128          # 16 k-tiles
CROW = D + 2 * EH      # 544: gather-table row [v(512) | ca(16) | cl(16)]
TWO_PI = 2.0 * math.pi
HG = 4                 # heads per pipeline group


def build_program():
    nc = bacc.Bacc()

    # ---------------- external I/O ----------------
    tT = nc.dram_tensor("tT", [D, LC], F32, kind="ExternalInput")        # target^T
    cT = nc.dram_tensor("cT", [D, K], F32, kind="ExternalInput")         # context^T
    Wt_d = nc.dram_tensor("Wt", [D, D], F32, kind="ExternalInput")
    Wcb_d = nc.dram_tensor("Wcb", [D, D], BF16, kind="ExternalInput")
    cTb_d = nc.dram_tensor("cTb", [D, K], BF16, kind="ExternalInput")
    Waclb_d = nc.dram_tensor("Waclb", [D, 32], BF16, kind="ExternalInput")
    Wo_d = nc.dram_tensor("Wo", [D, D], F32, kind="ExternalInput")
    bpack_d = nc.dram_tensor("bpack", [128, D], F32, kind="ExternalInput")  # bt@0|bc@32|bo@64
    Wtr3_d = nc.dram_tensor("Wtr3", [D, 48], F32, kind="ExternalInput")  # [Ws1t|Wa1t|Wl1t]
    Ws1c_d = nc.dram_tensor("Ws1c", [D, EH], F32, kind="ExternalInput")
    bs1_d = nc.dram_tensor("bs1", [1, EH], F32, kind="ExternalInput")
    bacl_d = nc.dram_tensor("bacl", [1, 32], F32, kind="ExternalInput")  # [ba1|bl1]
    Ws2bd_d = nc.dram_tensor("Ws2bd", [128, 8], F32, kind="ExternalInput")
    Wa2_d = nc.dram_tensor("Wa2", [1, EH], F32, kind="ExternalInput")
    ba2_d = nc.dram_tensor("ba2", [1, 1], F32, kind="ExternalInput")
    Wl2_d = nc.dram_tensor("Wl2", [128, H * R * HD], BF16, kind="ExternalInput")
    stp_d = nc.dram_tensor("stp", [128, T * NT], F32, kind="ExternalInput")
    lcol_d = nc.dram_tensor("lcol", [128, NT], F32, kind="ExternalInput")
    invf_d = nc.dram_tensor("invf", [1, HD // 2], F32, kind="ExternalInput")
    y_d = nc.dram_tensor("y", [LC, D], F32, kind="ExternalOutput")

    # internal DRAM gather table
    Tctx = nc.dram_tensor("Tctx", [K, CROW], BF16)

    # ---------------- persistent SBUF (static allocs, before pools) ----------
    ident = nc.alloc_sbuf_tensor("ident", [128, 128], F32).ap()
    ones1 = nc.alloc_sbuf_tensor("ones1", [128, 512], F32).ap()
    u_sb = [nc.alloc_sbuf_tensor(f"u{i}", [128, D], F32).ap() for i in range(NT)]
    trio = [nc.alloc_sbuf_tensor(f"trio{i}", [128, 48], F32).ap() for i in range(NT)]
    Wl2_sb = nc.alloc_sbuf_tensor("Wl2sb", [128, H * R * HD], BF16).ap()
    cpTrep = nc.alloc_sbuf_tensor("cpTrep", [128, K], F32).ap()
    tpbT = nc.alloc_sbuf_tensor("tpbT", [128, NT * 16], F32).ap()
    invf_sb = nc.alloc_sbuf_tensor("invfsb", [128, HD // 2], F32).ap()
    wa2_sb = nc.alloc_sbuf_tensor("wa2sb", [128, EH], F32).ap()
    ba2_sb = nc.alloc_sbuf_tensor("ba2sb", [128, 1], F32).ap()
    stp_sb = nc.alloc_sbuf_tensor("stpsb", [128, T * NT], F32).ap()
    stpn_sb = nc.alloc_sbuf_tensor("stpnsb", [128, T * NT], F32).ap()
    lcol_sb = nc.alloc_sbuf_tensor("lcolsb", [128, NT], F32).ap()
    bs1_sb = nc.alloc_sbuf_tensor("bs1sb", [1, EH], F32).ap()
    bacl_sb = nc.alloc_sbuf_tensor("baclsb", [1, 32], F32).ap()
    bpack_sb = nc.alloc_sbuf_tensor("bpacksb", [128, D], F32).ap()
    Ws2bd_sb = nc.alloc_sbuf_tensor("ws2bdsb", [128, 8], F32).ap()
    Wtr3_sb = nc.alloc_sbuf_tensor("wtr3sb", [128, 4 * 48], F32).ap()
    Waclb_sb = nc.alloc_sbuf_tensor("waclsb", [128, 4 * 32], BF16).ap()
    halfpi = nc.alloc_sbuf_tensor("halfpi", [128, 1], F32).ap()
    onec = nc.alloc_sbuf_tensor("onec", [128, 1], F32).ap()
    onesb = nc.alloc_sbuf_tensor("onesb", [1, 256], BF16).ap()

    with tile.TileContext(nc) as tc:
        with (
            tc.tile_pool(name="ld", bufs=2) as ldp,             # small staging tiles
            tc.tile_pool(name="gbp", bufs=2) as gbp,            # gather block
            tc.tile_pool(name="lamp", bufs=HG) as lamp,         # Lam (one per group head)
            tc.tile_pool(name="prodp", bufs=2) as prodp,        # einsum products
            tc.tile_pool(name="med", bufs=2) as medp,
            tc.tile_pool(name="sml", bufs=2) as smlp,
            tc.tile_pool(name="wp", bufs=1) as wp,
            tc.tile_pool(name="ps", bufs=2, space="PSUM") as psp,
            tc.tile_pool(name="ps4", bufs=4, space="PSUM") as ps4p,
        ):
            # ---------- constants ----------
            make_identity(nc, ident)
            nc.vector.memset(ones1, 1.0)
            nc.vector.memset(halfpi, math.pi / 2)
            nc.vector.memset(onec, 1.0)
            nc.vector.memset(onesb, 1.0)
            nc.sync.dma_start(out=invf_sb, in_=invf_d[:].partition_broadcast(128))
            nc.sync.dma_start(out=wa2_sb, in_=Wa2_d[:].partition_broadcast(128))
            nc.sync.dma_start(out=ba2_sb, in_=ba2_d[:].partition_broadcast(128))
            nc.sync.dma_start(out=lcol_sb, in_=lcol_d[:])
            nc.sync.dma_start(out=bs1_sb, in_=bs1_d[:])
            nc.sync.dma_start(out=bacl_sb, in_=bacl_d[:])
            nc.sync.dma_start(out=bpack_sb, in_=bpack_d[:])
            nc.sync.dma_start(out=Ws2bd_sb, in_=Ws2bd_d[:])
            nc.sync.dma_start(out=Wl2_sb, in_=Wl2_d[:])
            for dc in range(4):
                sl = slice(dc * 128, (dc + 1) * 128)
                nc.sync.dma_start(out=Wtr3_sb[:, dc * 48:(dc + 1) * 48], in_=Wtr3_d[sl, :])
                nc.sync.dma_start(out=Waclb_sb[:, dc * 32:(dc + 1) * 32], in_=Waclb_d[sl, :])

            bt_b = bpack_sb[0:1, :]
            bc_b = bpack_sb[32:33, :]
            bo_b = bpack_sb[64:65, :]

            def load_w(dram):
                t = wp.tile([128, 4 * D], F32, tag="wrhs")
                for dc in range(4):
                    nc.sync.dma_start(out=t[:, dc * D:(dc + 1) * D],
                                      in_=dram[dc * 128:(dc + 1) * 128, :])
                return t

            def softplus(dst, src, bias_ap, tmp_pool, tmp_tag):
                """dst = softplus(src + bias) = relu(x) + ln(1+exp(-|x|)).
                No softplus HW table; composed from abs/exp/ln (one table set)."""
                shp = [src.shape[0], src.free_size()]
                a = tmp_pool.tile(shp, F32, tag=tmp_tag)
                if bias_ap is None:
                    nc.scalar.activation(a[:], src, ACTF.Abs)
                    nc.vector.tensor_scalar(dst, src, 0.0, scalar2=None, op0=ALU.max)
                else:
                    nc.scalar.activation(a[:], src, ACTF.Abs, bias=bias_ap)
                    nc.vector.tensor_scalar(dst, src, bias_ap, scalar2=0.0,
                                            op0=ALU.add, op1=ALU.max)
                nc.scalar.activation(a[:], a[:], ACTF.Exp, scale=-1.0)
                nc.scalar.activation(a[:], a[:], ACTF.Ln, bias=onec[:, 0:1])
                nc.vector.tensor_tensor(dst, dst, a[:], op=ALU.add)

            stp_raw = smlp.tile([128, T * NT], F32, tag="stpraw")
            nc.sync.dma_start(out=stp_raw[:], in_=stp_d[:])
            softplus(stp_sb, stp_raw[:], None, smlp, "sptmp")
            nc.vector.tensor_scalar_mul(stpn_sb, stp_sb, -1.0)

            # ---------- dense projections ----------
            def mm_rows(out_ap, lhsT_dram, tix, w_sb, ncol, bias_sb, evac="v"):
                """out[128 rows of tile tix, ncol] = lhsT_dram[:, tile].T @ W (+ bias)."""
                ps = psp.tile([128, 512], F32, space="PSUM", tag="mmps")
                have_bias = bias_sb is not None
                if have_bias:
                    bb = bias_sb.base_partition()
                    nc.tensor.matmul(ps[:, :ncol], ones1[bb:bb + 1, :128],
                                     bias_sb[:1, :ncol], start=True, stop=False)
                for dc in range(4):
                    lh = ldp.tile([128, 128], F32, tag="lhst")
                    nc.sync.dma_start(
                        out=lh[:], in_=lhsT_dram[dc * 128:(dc + 1) * 128,
                                                 tix * 128:(tix + 1) * 128])
                    nc.tensor.matmul(ps[:, :ncol], lh[:],
                                     w_sb[:, dc * ncol:(dc + 1) * ncol],
                                     start=(not have_bias and dc == 0),
                                     stop=(dc == 3))
                if evac == "v":
                    nc.scalar.copy(out_ap, ps[:, :ncol])
                else:  # DRAM destination: stage through SBUF (DMA can't read PSUM)
                    stg = ldp.tile([128, 512], BF16, tag="stgb")
                    nc.scalar.copy(stg[:, :ncol], ps[:, :ncol])
                    nc.sync.dma_start(out=out_ap, in_=stg[:, :ncol])

            # cpT [16, K] = Ws1c.T @ context^T + bs1, then replicate 8x on
            # partitions (emitted first so the lt=0 score phase starts early)
            cpT_t = smlp.tile([EH, K], F32, tag="cpTt", bufs=1)
            cpT = cpT_t[:]
            for nt4 in range(4):
                nsl = slice(nt4 * 512, (nt4 + 1) * 512)
                ps = psp.tile([128, 512], F32, space="PSUM", tag="mmps")
                nc.tensor.matmul(ps[:EH, :], bs1_sb[:1, :], ones1[:1, :512],
                                 start=True, stop=False)
                for dc in range(4):
                    lh = ldp.tile([128, EH], F32, tag="lhst16")
                    nc.sync.dma_start(out=lh[:],
                                      in_=Ws1c_d[dc * 128:(dc + 1) * 128, :])
                    rh = ldp.tile([128, 512], F32, tag="ctchunk")
                    nc.sync.dma_start(out=rh[:], in_=cT[dc * 128:(dc + 1) * 128, nsl])
                    nc.tensor.matmul(ps[:EH, :], lh[:], rh[:],
                                     start=False, stop=(dc == 3))
                nc.vector.tensor_copy(cpT[:, nsl], ps[:EH, :])
            for ls in range(8):
                nc.sync.dma_start(out=cpTrep[ls * 16:(ls + 1) * 16, :], in_=cpT[:, :])

            # fused u + trio projections (one lhsT load per chunk)
            Wt_t = load_w(Wt_d)
            for lt in range(NT):
                psu = psp.tile([128, 512], F32, space="PSUM", tag="mmps")
                pst3 = ps4p.tile([128, 512], F32, space="PSUM", tag="lamps")
                nc.tensor.matmul(psu[:, :], ones1[0:1, :128], bt_b[:1, :],
                                 start=True, stop=False)
                for dc in range(4):
                    lh = ldp.tile([128, 128], F32, tag="lhst")
                    nc.sync.dma_start(
                        out=lh[:], in_=tT[dc * 128:(dc + 1) * 128,
                                          lt * 128:(lt + 1) * 128])
                    nc.tensor.matmul(psu[:, :], lh[:],
                                     Wt_t[:, dc * D:(dc + 1) * D],
                                     start=False, stop=(dc == 3))
                    nc.tensor.matmul(pst3[:, :48], lh[:],
                                     Wtr3_sb[:, dc * 48:(dc + 1) * 48],
                                     start=(dc == 0), stop=(dc == 3))
                nc.scalar.copy(u_sb[lt][:], psu[:, :])
                nc.scalar.copy(trio[lt][:], pst3[:, :48])
                # tpbT: per-octet score bias columns, partition p = ls*16 + e
                for oc in range(16):
                    nc.sync.dma_start(
                        out=tpbT[:, lt * 16 + oc:lt * 16 + oc + 1],
                        in_=trio[lt][oc * 8:(oc + 1) * 8, 0:EH])

            # context projection -> Tctx, bf16 single-pass matmuls, fused loads
            Wcb_t = wp.tile([128, 4 * D], BF16, tag="wrhsb")
            for dc in range(4):
                nc.sync.dma_start(out=Wcb_t[:, dc * D:(dc + 1) * D],
                                  in_=Wcb_d[dc * 128:(dc + 1) * 128, :])
            for kt in range(KT):
                psv = psp.tile([128, 512], F32, space="PSUM", tag="mmps")
                psa = ps4p.tile([128, 512], F32, space="PSUM", tag="lamps")
                nc.tensor.matmul(psv[:, :], ones1[32:33, :128], bc_b[:1, :],
                                 start=True, stop=False)
                nc.tensor.matmul(psa[:, :32], ones1[0:1, :128], bacl_sb[:1, :],
                                 start=True, stop=False)
                for dc in range(4):
                    lh = ldp.tile([128, 128], BF16, tag="lhstb")
                    nc.sync.dma_start(
                        out=lh[:], in_=cTb_d[dc * 128:(dc + 1) * 128,
                                            kt * 128:(kt + 1) * 128])
                    nc.tensor.matmul(psv[:, :], lh[:],
                                     Wcb_t[:, dc * D:(dc + 1) * D],
                                     start=False, stop=(dc == 3))
                    nc.tensor.matmul(psa[:, :32], lh[:],
                                     Waclb_sb[:, dc * 32:(dc + 1) * 32],
                                     start=False, stop=(dc == 3))
                stg = ldp.tile([128, CROW], BF16, tag="stgb")
                nc.scalar.copy(stg[:, 0:D], psv[:, :])
                nc.scalar.copy(stg[:, D:D + 32], psa[:, :32])
                nc.sync.dma_start(out=Tctx[kt * 128:(kt + 1) * 128, :],
                                  in_=stg[:, :])

            # ---------- per l-tile ----------
            for lt in range(NT):
                # ----- scores + top-8 -----
                scores = medp.tile([128, K], F32, tag="scores", bufs=1)
                for oc in range(16):
                    for hf in range(2):
                        g_sc = medp.tile([128, K // 2], F32, tag="gsc", bufs=1)
                        nc.scalar.activation(
                            g_sc[:], cpTrep[:, hf * 1024:(hf + 1) * 1024], ACTF.Gelu,
                            bias=tpbT[:, lt * 16 + oc:lt * 16 + oc + 1])
                        for nq in range(2):
                            col = hf * 1024 + nq * 512
                            pssc = psp.tile([8, 512], F32, space="PSUM", tag="small")
                            nc.tensor.matmul(pssc[:, :], Ws2bd_sb[:],
                                             g_sc[:, nq * 512:(nq + 1) * 512],
                                             start=True, stop=True)
                            sstg = medp.tile([8, 512], F32, tag="sstg")
                            nc.scalar.copy(sstg[:], pssc[:, :])
                            nc.sync.dma_start(
                                out=scores[oc * 8:(oc + 1) * 8, col:col + 512],
                                in_=sstg[:])
                mx8 = smlp.tile([128, 8], F32, tag="mx8")
                idx = smlp.tile([128, 8], U32, tag="idx")
                nc.vector.max(out=mx8[:], in_=scores[:])
                nc.vector.max_index(out=idx[:], in_max=mx8[:], in_values=scores[:])

                # ----- gather context-side rows -----
                gb = gbp.tile([128, WWIN * CROW], BF16, tag="gb")
                gbv = gb[:].rearrange("p (w c) -> p w c", w=8)
                for w in range(WWIN):
                    nc.gpsimd.indirect_dma_start(
                        out=gb[:, w * CROW:(w + 1) * CROW],
                        out_offset=None,
                        in_=Tctx[:, :],
                        in_offset=bass.IndirectOffsetOnAxis(ap=idx[:, w:w + 1], axis=0),
                    )

                # ----- per-edge angles -----
                jf = smlp.tile([128, 8], F32, tag="jf")
                nc.vector.tensor_copy(jf[:], idx[:])
                delta = smlp.tile([128, 8], F32, tag="delta")
                nc.vector.tensor_scalar(delta[:], jf[:], lcol_sb[:, lt:lt + 1],
                                        scalar2=None, op0=ALU.subtract)
                ang = medp.tile([128, 8 * 32], F32, tag="ang", bufs=1)
                nc.vector.tensor_tensor(
                    out=ang[:].rearrange("p (w f) -> p w f", w=8),
                    in0=delta[:].unsqueeze(2).to_broadcast((128, 8, 32)),
                    in1=invf_sb[:].unsqueeze(1).to_broadcast((128, 8, 32)),
                    op=ALU.mult)
                # range-reduce to [-pi, pi]: x - 2pi*round(x/2pi), round via
                # the +/- 1.5*2^23 magic-number trick (no mod/floor on DVE ISA)
                MAGIC = 1.5 * 2.0 ** 23
                angt = medp.tile([128, 8 * 32], F32, tag="angt", bufs=1)
                nc.vector.tensor_scalar_mul(angt[:], ang[:], 1.0 / TWO_PI)
                angr = medp.tile([128, 8 * 32], F32, tag="angr", bufs=1)
                nc.vector.tensor_scalar(angr[:], angt[:], MAGIC, scalar2=MAGIC,
                                        op0=ALU.add, op1=ALU.subtract)
                nc.vector.tensor_sub(angt[:], angt[:], angr[:])
                nc.vector.tensor_scalar_mul(ang[:], angt[:], TWO_PI)
                cosb = medp.tile([128, 8 * 32], F32, tag="cosb")
                sinb = medp.tile([128, 8 * 32], F32, tag="sinb")
                nc.scalar.activation(sinb[:], ang[:], ACTF.Sin, scale=-1.0)
                nc.vector.tensor_scalar_mul(angr[:], ang[:], -1.0)
                nc.vector.tensor_max(angt[:], ang[:], angr[:])
                nc.scalar.activation(cosb[:], angt[:], ACTF.Sin, scale=-1.0,
                                     bias=halfpi[:, 0:1])
                # sinb2[w, a, f]: a=0 -> -sin (pairs with -u_hi), a=1 -> +sin
                sinb2 = medp.tile([128, 8 * 2 * 32], F32, tag="sinb2")
                s2v = sinb2[:].rearrange("p (w a f) -> p w a f", w=8, a=2)
                nc.vector.tensor_scalar_mul(
                    s2v[:, :, 0, :], sinb[:].rearrange("p (w f) -> p w f", w=8),
                    -1.0)
                nc.vector.tensor_copy(
                    s2v[:, :, 1, :], sinb[:].rearrange("p (w f) -> p w f", w=8))

                # ----- alphas = softplus(gelu(ta+ca) @ Wa2 + ba2) -----
                ha = smlp.tile([128, 8 * EH], F32, tag="ha")
                nc.vector.tensor_tensor(
                    out=ha[:].rearrange("p (w c) -> p w c", w=8),
                    in0=trio[lt][:, 16:32].unsqueeze(1).to_broadcast((128, 8, EH)),
                    in1=gbv[:, :, D:D + EH],
                    op=ALU.add)
                nc.scalar.activation(ha[:], ha[:], ACTF.Gelu)
                haw = smlp.tile([128, 8 * EH], F32, tag="haw")
                nc.vector.tensor_tensor(
                    out=haw[:].rearrange("p (w c) -> p w c", w=8),
                    in0=ha[:].rearrange("p (w c) -> p w c", w=8),
                    in1=wa2_sb[:].unsqueeze(1).to_broadcast((128, 8, EH)),
                    op=ALU.mult)
                alphas = smlp.tile([128, 8], F32, tag="alphas")
                nc.vector.tensor_reduce(alphas[:], haw[:].rearrange(
                    "p (w c) -> p w c", w=8), axis=AX.X, op=ALU.add)
                softplus(alphas[:], alphas[:], ba2_sb[:, 0:1], smlp, "sptmp")
                # alpharep [128, (w d)] = alphas replicated over d (t/h-invariant)
                alpharep = smlp.tile([128, WWIN * HD], F32, tag="alpharep")
                nc.vector.tensor_copy(
                    alpharep[:].rearrange("p (w d) -> p w d", w=8),
                    alphas[:].unsqueeze(2).to_broadcast((128, 8, HD)))

                # ----- g = gelu(tl + cl) and per-w transposes -----
                gmat = smlp.tile([128, 8 * EH], F32, tag="gmat")
                nc.vector.tensor_tensor(
                    out=gmat[:].rearrange("p (w c) -> p w c", w=8),
                    in0=trio[lt][:, 32:48].unsqueeze(1).to_broadcast((128, 8, EH)),
                    in1=gbv[:, :, D + EH:D + 2 * EH],
                    op=ALU.add)
                nc.scalar.activation(gmat[:], gmat[:], ACTF.Gelu)
                gT4 = gbp.tile([128, 2 * 128], BF16, tag="gT4")  # 2 quads side by side
                nc.vector.memset(gT4[:], 0.0)
                for s4 in range(4):  # bias row (constant 1) for the bl2 fold
                    nc.sync.dma_start(out=gT4[32 * s4 + EH:32 * s4 + EH + 1, :],
                                      in_=onesb[:1, 0:256])
                for w in range(WWIN):
                    q, s = w // 4, w % 4
                    pst = psp.tile([EH, 128], F32, space="PSUM", tag="small")
                    nc.tensor.transpose(
                        out=pst[:, :],
                        in_=gmat[:].rearrange("p (w c) -> p w c", w=8)[:, w, :],
                        identity=ident)
                    nc.vector.tensor_copy(
                        gT4[32 * s:32 * s + EH, q * 128:(q + 1) * 128], pst[:, :])

                # ----- 4-head pipeline groups -----
                for hg in range(H // HG):
                    heads = range(hg * HG, (hg + 1) * HG)
                    lam_h = {}
                    rec_h = {}
                    for h in heads:
                        # Lam layout: (w, r, d) bf16, contiguous
                        Lam = lamp.tile([128, WWIN * R * HD], BF16, tag="lam")
                        lam_h[h] = Lam
                        for w in range(WWIN):
                            q, s = w // 4, w % 4
                            psl = ps4p.tile([128, 512], F32, space="PSUM", tag="lamps")
                            nc.tensor.matmul(
                                psl[:, :], gT4[32 * s:32 * s + 32, q * 128:(q + 1) * 128],
                                Wl2_sb[32 * s:32 * s + 32, h * R * HD:(h + 1) * R * HD],
                                start=True, stop=True, tile_position=(32 * s, 0))
                            nc.scalar.copy(
                                Lam[:, w * R * HD:(w + 1) * R * HD], psl[:, :])
                        # squared row norms, fused across w: sq = Lam^2 (ACT)
                        sq = prodp.tile([128, WWIN * R * HD], BF16, tag="prodb", bufs=3)
                        nc.scalar.activation(sq[:], Lam[:], ACTF.Square)
                        n2 = smlp.tile([128, WWIN * R], F32, tag="n2")
                        nc.vector.tensor_reduce(
                            n2[:].rearrange("p (w r) -> p w r", w=8),
                            sq[:].rearrange("p (w r d) -> p w r d", w=8, r=R),
                            axis=AX.X, op=ALU.add)
                        nrm = smlp.tile([128, WWIN * R], F32, tag="nrm")
                        nc.vector.tensor_scalar_max(nrm[:], n2[:], 1e-24)
                        rec = smlp.tile([128, WWIN * R], F32, tag="rec", bufs=HG)
                        rec_h[h] = rec
                        nc.vector.reciprocal(rec[:], nrm[:])

                    usl = {h: u_sb[lt][:, h * HD:(h + 1) * HD] for h in heads}

                    def emit_diff_e1p(h, t):
                        lam4 = lam_h[h][:].rearrange(
                            "p (w r d) -> p w r d", w=8, r=R)
                        # diff = u_i*cos + rot_half(u_i)*sin - v_j   [128, (w,d)]
                        diff = medp.tile([128, WWIN * HD], BF16, tag="diff",
                                         bufs=HG)
                        d3 = diff[:].rearrange("p (w d) -> p w d", w=8)
                        t0 = medp.tile([128, WWIN * HD], BF16, tag="t0", bufs=3)
                        nc.vector.tensor_tensor(
                            out=t0[:].rearrange("p (w a b) -> p w a b", w=8, a=2),
                            in0=usl[h].rearrange("p (a b) -> p a b", a=2)
                                .unsqueeze(1).to_broadcast((128, 8, 2, 32)),
                            in1=cosb[:].rearrange("p (w f) -> p w f", w=8)
                                .unsqueeze(2).to_broadcast((128, 8, 2, 32)),
                            op=ALU.mult)
                        t1 = medp.tile([128, WWIN * HD], BF16, tag="t0", bufs=3)
                        nc.vector.tensor_tensor(
                            out=t1[:].rearrange("p (w a b) -> p w a b", w=8, a=2),
                            in0=usl[h].rearrange("p (a b) -> p a b", a=2)[:, ::-1, :]
                                .unsqueeze(1).to_broadcast((128, 8, 2, 32)),
                            in1=s2v,
                            op=ALU.mult)
                        nc.vector.tensor_tensor(out=t0[:], in0=t0[:], in1=t1[:],
                                                op=ALU.add)
                        nc.vector.tensor_tensor(
                            out=d3,
                            in0=t0[:].rearrange("p (w d) -> p w d", w=8),
                            in1=gbv[:, :, h * HD:(h + 1) * HD],
                            op=ALU.subtract)
                        # einsum1 products on GPSIMD (overlaps DVE)
                        prod = prodp.tile([128, WWIN * R * HD], BF16, tag="prodb",
                                          bufs=3)
                        nc.gpsimd.tensor_tensor(
                            out=prod[:].rearrange("p (w r d) -> p w r d", w=8, r=R),
                            in0=lam4,
                            in1=d3.unsqueeze(2).to_broadcast((128, 8, R, HD)),
                            op=ALU.mult)
                        return diff, prod

                    def emit_e1r_e2p(h, t, prod):
                        lam4 = lam_h[h][:].rearrange(
                            "p (w r d) -> p w r d", w=8, r=R)
                        # einsum1 reduce: Ld[w,r] = sum_d Lam*diff
                        ld = smlp.tile([128, WWIN * R], F32, tag="ld", bufs=3)
                        nc.vector.tensor_reduce(
                            ld[:].rearrange("p (w r) -> p w r", w=8),
                            prod[:].rearrange("p (w r d) -> p w r d", w=8, r=R),
                            axis=AX.X, op=ALU.add)
                        ld2 = smlp.tile([128, WWIN * R], F32, tag="ld2", bufs=3)
                        nc.vector.tensor_tensor(ld2[:], ld[:], rec_h[h][:],
                                                op=ALU.mult)
                        # einsum2 products on GPSIMD: prod2 (w, d, r)
                        prod2 = prodp.tile([128, WWIN * HD * R], BF16, tag="prodb2",
                                           bufs=2)
                        nc.gpsimd.tensor_tensor(
                            out=prod2[:].rearrange("p (w d r) -> p w d r",
                                                   w=8, d=HD),
                            in0=lam4.transpose([0, 1, 3, 2]),
                            in1=ld2[:].rearrange("p (w r) -> p w r", w=8)
                                .unsqueeze(2).to_broadcast((128, 8, HD, R)),
                            op=ALU.mult)
                        return prod2

                    def emit_adiff(h, t, diff):
                        adiff = smlp.tile([128, WWIN * HD], BF16, tag="adiff",
                                          bufs=3)
                        nc.vector.tensor_tensor(adiff[:], diff[:], alpharep[:],
                                                op=ALU.mult)
                        return adiff

                    def emit_tail(h, t, prod2, adiff):
                        stc = slice(t * NT + lt, t * NT + lt + 1)
                        # einsum2 reduce over r, + alpha*diff, then sum over w
                        res = smlp.tile([128, WWIN * HD], F32, tag="res")
                        nc.vector.tensor_reduce(
                            res[:].rearrange("p (w d) -> p w d", w=8),
                            prod2[:].rearrange("p (w d r) -> p w d r",
                                               w=8, d=HD),
                            axis=AX.X, op=ALU.add)
                        nc.vector.tensor_tensor(res[:], res[:], adiff[:],
                                                op=ALU.add)
                        rsum = smlp.tile([128, HD], F32, tag="rsum")
                        nc.vector.tensor_reduce(
                            rsum[:],
                            res[:].rearrange("p (w d) -> p d w", w=8),
                            axis=AX.X, op=ALU.add)
                        # u -= step*rsum
                        nc.vector.scalar_tensor_tensor(
                            out=usl[h], in0=rsum[:], scalar=stpn_sb[:, stc],
                            in1=usl[h], op0=ALU.mult, op1=ALU.add)

                    # software-pipelined 2-iteration t-loop
                    st = {}
                    for h in heads:
                        st[h] = emit_diff_e1p(h, 0)
                    prod2s = {}
                    for h in heads:
                        prod2s[h] = emit_e1r_e2p(h, 0, st[h][1])
                    adiffs = {}
                    for h in heads:
                        adiffs[h] = emit_adiff(h, 0, st[h][0])
                    for h in heads:
                        emit_tail(h, 0, prod2s[h], adiffs[h])
                        st[h] = emit_diff_e1p(h, 1)
                    for h in heads:
                        prod2s[h] = emit_e1r_e2p(h, 1, st[h][1])
                    for h in heads:
                        adiffs[h] = emit_adiff(h, 1, st[h][0])
                    for h in heads:
                        emit_tail(h, 1, prod2s[h], adiffs[h])

            # ---------- output projection: y = u @ Wo + bo ----------
            Wo_t = load_w(Wo_d)
            for lt in range(NT):
                psy = psp.tile([128, 512], F32, space="PSUM", tag="mmps")
                nc.tensor.matmul(psy[:, :], ones1[64:65, :128], bo_b[:1, :],
                                 start=True, stop=False)
                for dc in range(4):
                    pst = psp.tile([128, 128], F32, space="PSUM", tag="small")
                    nc.tensor.transpose(
                        out=pst[:, :], in_=u_sb[lt][:, dc * 128:(dc + 1) * 128],
                        identity=ident)
                    uT = ldp.tile([128, 128], F32, tag="uT")
                    nc.scalar.copy(uT[:], pst[:, :])
                    nc.tensor.matmul(psy[:, :], uT[:], Wo_t[:, dc * D:(dc + 1) * D],
                                     start=False, stop=(dc == 3))
                ystg = ldp.tile([128, 512], F32, tag="stg")
                nc.scalar.copy(ystg[:], psy[:, :])
                nc.sync.dma_start(out=y_d[lt * 128:(lt + 1) * 128, :], in_=ystg[:])

    nc.finalize()
    return nc


def make_in_maps(inputs):
    """Host-side prep: slice/transpose inputs into the 8 per-core input maps."""
    target = np.asarray(inputs["target"], np.float32)
    context = np.asarray(inputs["context"], np.float32)
    Wt = np.asarray(inputs["Wt"], np.float32)
    bt = np.asarray(inputs["bt"], np.float32)
    Wc = np.asarray(inputs["Wc"], np.float32)
    bc = np.asarray(inputs["bc"], np.float32)
    Ws1 = np.asarray(inputs["Ws1"], np.float32)
    bs1 = np.asarray(inputs["bs1"], np.float32)
    Ws2 = np.asarray(inputs["Ws2"], np.float32)
    Wa1 = np.asarray(inputs["Wa1"], np.float32)
    ba1 = np.asarray(inputs["ba1"], np.float32)
    Wa2 = np.asarray(inputs["Wa2"], np.float32)
    ba2 = np.asarray(inputs["ba2"], np.float32)
    Wl1 = np.asarray(inputs["Wl1"], np.float32)
    bl1 = np.asarray(inputs["bl1"], np.float32)
    Wl2 = np.asarray(inputs["Wl2"], np.float32)
    bl2 = np.asarray(inputs["bl2"], np.float32)
    step_sizes = np.asarray(inputs["step_sizes"], np.float32)
    Wo = np.asarray(inputs["Wo"], np.float32)
    bo = np.asarray(inputs["bo"], np.float32)

    import ml_dtypes
    Ws2bd = np.zeros((128, 8), np.float32)  # cast to bf16 below
    for ls in range(8):
        Ws2bd[ls * 16:(ls + 1) * 16, ls] = Ws2[:, 0]
    Wl2s = np.zeros((128, H * R * HD), np.float32)
    for s in range(4):
        Wl2s[32 * s:32 * s + EH, :] = Wl2
        Wl2s[32 * s + EH, :] = bl2
    Wl2s = Wl2s.astype(ml_dtypes.bfloat16)
    invf = (1.0 / (10000.0 ** (np.arange(0, HD, 2, dtype=np.float32) / HD)))[None, :]
    bpack = np.zeros((128, D), np.float32)
    bpack[0] = bt
    bpack[32] = bc
    bpack[64] = bo

    common = dict(
        Wt=Wt, Wcb=Wc.astype(ml_dtypes.bfloat16), Wo=Wo, bpack=bpack,
        Wtr3=np.ascontiguousarray(np.concatenate([Ws1[:D], Wa1[:D], Wl1[:D]], axis=1)),
        Ws1c=np.ascontiguousarray(Ws1[D:]),
        Waclb=np.ascontiguousarray(
            np.concatenate([Wa1[D:], Wl1[D:]], axis=1)).astype(ml_dtypes.bfloat16),
        bs1=bs1[None, :],
        bacl=np.concatenate([ba1, bl1])[None, :],
        Ws2bd=Ws2bd, Wa2=np.ascontiguousarray(Wa2.T),
        ba2=np.asarray(ba2, np.float32).reshape(1, 1), Wl2=Wl2s,
        invf=np.ascontiguousarray(invf, np.float32),
    )

    in_maps = []
    for c in range(8):
        b, rc = c // 4, c % 4
        rows = slice(rc * LC, (rc + 1) * LC)
        stp = np.ascontiguousarray(
            step_sizes[:, rows].reshape(T, NT, 128).transpose(2, 0, 1)
            .reshape(128, T * NT))
        lcol = np.ascontiguousarray(
            (rc * LC + np.arange(LC, dtype=np.float32)).reshape(NT, 128).T)
        m = dict(common)
        cTf = np.ascontiguousarray(context[b].T)
        m.update(
            tT=np.ascontiguousarray(target[b, rows].T),
            cT=cTf, cTb=cTf.astype(ml_dtypes.bfloat16),
            stp=stp, lcol=lcol,
        )
        in_maps.append(m)
    return in_maps


_NC_CACHE = {}


def kernel(**inputs):
    if "nc" not in _NC_CACHE:
        _NC_CACHE["nc"] = build_program()
    nc = _NC_CACHE["nc"]
    in_maps = make_in_maps(inputs)
    res = run_bass_kernel_spmd(nc, in_maps, list(range(8)))
    out = np.empty((B, L, D), np.float32)
    for c in range(8):
        b, rc = c // 4, c % 4
        out[b, rc * LC:(rc + 1) * LC] = res.results[c]["y"]
    return out


# revision 19
# speedup vs baseline: 1.3872x; 1.0067x over previous
"""CrossConsensus kernel for 8 Trainium2 NeuronCores.

Sharding: data-parallel over B*L rows. Core c handles batch b=c//4,
target rows [ (c%4)*512, (c%4+1)*512 ).  All computation is row-local
(edge_i = repeat(arange(L), 8) means each edge scatters back to its own
source row), so there are no collectives; each core needs its target
row-chunk plus the full context of its batch.

v3: r8 einsums (alpha via per-l-tile alpharep), einsum products on
GPSIMD, fused across-w norms, ACT-engine PSUM evacuations, no uR state
(rotate_half via a reversed-stride view of u and sign-folded sin), and
4-head pipeline groups so DVE keeps working while GPSIMD computes.
"""

import math

import numpy as np

import concourse.bass as bass
import concourse.bacc as bacc
import concourse.tile as tile
from concourse import mybir
from concourse.bass_utils import run_bass_kernel_spmd
from concourse.masks import make_identity

F32 = mybir.dt.float32
BF16 = mybir.dt.bfloat16
U32 = mybir.dt.uint32
AX = mybir.AxisListType
ALU = mybir.AluOpType
ACTF = mybir.ActivationFunctionType

# problem constants (hardcoded per the harness contract)
B, L, K, D = 2, 2048, 2048, 512
H, R, WWIN, T, EH = 8, 8, 8, 2, 16
HD = D // H            # 64
LC = L * B // 8        # 512 rows per core
NT = LC // 128         # 4 l-tiles per core
KT = K // 128          # 16 k-tiles
CROW = D + 2 * EH      # 544: gather-table row [v(512) | ca(16) | cl(16)]
TWO_PI = 2.0 * math.pi
HG = 4                 # heads per pipeline group


def build_program():
    nc = bacc.Bacc()

    # ---------------- external I/O ----------------
    tT = nc.dram_tensor("tT", [D, LC], F32, kind="ExternalInput")        # target^T
    cT = nc.dram_tensor("cT", [D, K], F32, kind="ExternalInput")         # context^T
    Wt_d = nc.dram_tensor("Wt", [D, D], F32, kind="ExternalInput")
    Wcb_d = nc.dram_tensor("Wcb", [D, D], BF16, kind="ExternalInput")
    cTb_d = nc.dram_tensor("cTb", [D, K], BF16, kind="ExternalInput")
    Waclb_d = nc.dram_tensor("Waclb", [D, 32], BF16, kind="ExternalInput")
    Wo_d = nc.dram_tensor("Wo", [D, D], F32, kind="ExternalInput")
    bpack_d = nc.dram_tensor("bpack", [128, D], F32, kind="ExternalInput")  # bt@0|bc@32|bo@64
    Wtr3_d = nc.dram_tensor("Wtr3", [D, 48], F32, kind="ExternalInput")  # [Ws1t|Wa1t|Wl1t]
    Ws1c_d = nc.dram_tensor("Ws1c", [D, EH], F32, kind="ExternalInput")
    bs1_d = nc.dram_tensor("bs1", [1, EH], F32, kind="ExternalInput")
    bacl_d = nc.dram_tensor("bacl", [1, 32], F32, kind="ExternalInput")  # [ba1|bl1]
    Ws2bd_d = nc.dram_tensor("Ws2bd", [128, 8], F32, kind="ExternalInput")
    Wa2_d = nc.dram_tensor("Wa2", [1, EH], F32, kind="ExternalInput")
    ba2_d = nc.dram_tensor("ba2", [1, 1], F32, kind="ExternalInput")
    Wl2_d = nc.dram_tensor("Wl2", [128, H * R * HD], BF16, kind="ExternalInput")
    stp_d = nc.dram_tensor("stp", [128, T * NT], F32, kind="ExternalInput")
    lcol_d = nc.dram_tensor("lcol", [128, NT], F32, kind="ExternalInput")
    invf_d = nc.dram_tensor("invf", [1, HD // 2], F32, kind="ExternalInput")
    y_d = nc.dram_tensor("y", [LC, D], F32, kind="ExternalOutput")

    # internal DRAM gather table
    Tctx = nc.dram_tensor("Tctx", [K, CROW], BF16)

    # ---------------- persistent SBUF (static allocs, before pools) ----------
    ident = nc.alloc_sbuf_tensor("ident", [128, 128], F32).ap()
    ones1 = nc.alloc_sbuf_tensor("ones1", [128, 512], F32).ap()
    u_sb = [nc.alloc_sbuf_tensor(f"u{i}", [128, D], F32).ap() for i in range(NT)]
    trio = [nc.alloc_sbuf_tensor(f"trio{i}", [128, 48], F32).ap() for i in range(NT)]
    Wl2_sb = nc.alloc_sbuf_tensor("Wl2sb", [128, H * R * HD], BF16).ap()
    cpTrep = nc.alloc_sbuf_tensor("cpTrep", [128, K], F32).ap()
    tpbT = nc.alloc_sbuf_tensor("tpbT", [128, NT * 16], F32).ap()
    invf_sb = nc.alloc_sbuf_tensor("invfsb", [128, HD // 2], F32).ap()
    wa2_sb = nc.alloc_sbuf_tensor("wa2sb", [128, EH], F32).ap()
    ba2_sb = nc.alloc_sbuf_tensor("ba2sb", [128, 1], F32).ap()
    stp_sb = nc.alloc_sbuf_tensor("stpsb", [128, T * NT], F32).ap()
    stpn_sb = nc.alloc_sbuf_tensor("stpnsb", [128, T * NT], F32).ap()
    lcol_sb = nc.alloc_sbuf_tensor("lcolsb", [128, NT], F32).ap()
    bs1_sb = nc.alloc_sbuf_tensor("bs1sb", [1, EH], F32).ap()
    bacl_sb = nc.alloc_sbuf_tensor("baclsb", [1, 32], F32).ap()
    bpack_sb = nc.alloc_sbuf_tensor("bpacksb", [128, D], F32).ap()
    Ws2bd_sb = nc.alloc_sbuf_tensor("ws2bdsb", [128, 8], F32).ap()
    Wtr3_sb = nc.alloc_sbuf_tensor("wtr3sb", [128, 4 * 48], F32).ap()
    Waclb_sb = nc.alloc_sbuf_tensor("waclsb", [128, 4 * 32], BF16).ap()
    halfpi = nc.alloc_sbuf_tensor("halfpi", [128, 1], F32).ap()
    onec = nc.alloc_sbuf_tensor("onec", [128, 1], F32).ap()
    onesb = nc.alloc_sbuf_tensor("onesb", [1, 256], BF16).ap()

    with tile.TileContext(nc) as tc:
        with (
            tc.tile_pool(name="ld", bufs=2) as ldp,             # small staging tiles
            tc.tile_pool(name="gbp", bufs=2) as gbp,            # gather block
            tc.tile_pool(name="lamp", bufs=HG) as lamp,         # Lam (one per group head)
            tc.tile_pool(name="prodp", bufs=2) as prodp,        # einsum products
            tc.tile_pool(name="med", bufs=2) as medp,
            tc.tile_pool(name="sml", bufs=2) as smlp,
            tc.tile_pool(name="wp", bufs=1) as wp,
            tc.tile_pool(name="ps", bufs=2, space="PSUM") as psp,
            tc.tile_pool(name="ps4", bufs=4, space="PSUM") as ps4p,
        ):
            # ---------- constants ----------
            make_identity(nc, ident)
            nc.vector.memset(ones1, 1.0)
            nc.vector.memset(halfpi, math.pi / 2)
            nc.vector.memset(onec, 1.0)
            nc.vector.memset(onesb, 1.0)
            nc.sync.dma_start(out=invf_sb, in_=invf_d[:].partition_broadcast(128))
            nc.sync.dma_start(out=wa2_sb, in_=Wa2_d[:].partition_broadcast(128))
            nc.sync.dma_start(out=ba2_sb, in_=ba2_d[:].partition_broadcast(128))
            nc.sync.dma_start(out=lcol_sb, in_=lcol_d[:])
            nc.sync.dma_start(out=bs1_sb, in_=bs1_d[:])
            nc.sync.dma_start(out=bacl_sb, in_=bacl_d[:])
            nc.sync.dma_start(out=bpack_sb, in_=bpack_d[:])
            nc.sync.dma_start(out=Ws2bd_sb, in_=Ws2bd_d[:])
            nc.sync.dma_start(out=Wl2_sb, in_=Wl2_d[:])
            for dc in range(4):
                sl = slice(dc * 128, (dc + 1) * 128)
                nc.sync.dma_start(out=Wtr3_sb[:, dc * 48:(dc + 1) * 48], in_=Wtr3_d[sl, :])
                nc.sync.dma_start(out=Waclb_sb[:, dc * 32:(dc + 1) * 32], in_=Waclb_d[sl, :])

            bt_b = bpack_sb[0:1, :]
            bc_b = bpack_sb[32:33, :]
            bo_b = bpack_sb[64:65, :]

            def load_w(dram):
                t = wp.tile([128, 4 * D], F32, tag="wrhs")
                for dc in range(4):
                    nc.sync.dma_start(out=t[:, dc * D:(dc + 1) * D],
                                      in_=dram[dc * 128:(dc + 1) * 128, :])
                return t

            def softplus(dst, src, bias_ap, tmp_pool, tmp_tag):
                """dst = softplus(src + bias) = relu(x) + ln(1+exp(-|x|)).
                No softplus HW table; composed from abs/exp/ln (one table set)."""
                shp = [src.shape[0], src.free_size()]
                a = tmp_pool.tile(shp, F32, tag=tmp_tag)
                if bias_ap is None:
                    nc.scalar.activation(a[:], src, ACTF.Abs)
                    nc.vector.tensor_scalar(dst, src, 0.0, scalar2=None, op0=ALU.max)
                else:
                    nc.scalar.activation(a[:], src, ACTF.Abs, bias=bias_ap)
                    nc.vector.tensor_scalar(dst, src, bias_ap, scalar2=0.0,
                                            op0=ALU.add, op1=ALU.max)
                nc.scalar.activation(a[:], a[:], ACTF.Exp, scale=-1.0)
                nc.scalar.activation(a[:], a[:], ACTF.Ln, bias=onec[:, 0:1])
                nc.vector.tensor_tensor(dst, dst, a[:], op=ALU.add)

            stp_raw = smlp.tile([128, T * NT], F32, tag="stpraw")
            nc.sync.dma_start(out=stp_raw[:], in_=stp_d[:])
            softplus(stp_sb, stp_raw[:], None, smlp, "sptmp")
            nc.vector.tensor_scalar_mul(stpn_sb, stp_sb, -1.0)

            # ---------- dense projections ----------
            def mm_rows(out_ap, lhsT_dram, tix, w_sb, ncol, bias_sb, evac="v"):
                """out[128 rows of tile tix, ncol] = lhsT_dram[:, tile].T @ W (+ bias)."""
                ps = psp.tile([128, 512], F32, space="PSUM", tag="mmps")
                have_bias = bias_sb is not None
                if have_bias:
                    bb = bias_sb.base_partition()
                    nc.tensor.matmul(ps[:, :ncol], ones1[bb:bb + 1, :128],
                                     bias_sb[:1, :ncol], start=True, stop=False)
                for dc in range(4):
                    lh = ldp.tile([128, 128], F32, tag="lhst")
                    nc.sync.dma_start(
                        out=lh[:], in_=lhsT_dram[dc * 128:(dc + 1) * 128,
                                                 tix * 128:(tix + 1) * 128])
                    nc.tensor.matmul(ps[:, :ncol], lh[:],
                                     w_sb[:, dc * ncol:(dc + 1) * ncol],
                                     start=(not have_bias and dc == 0),
                                     stop=(dc == 3))
                if evac == "v":
                    nc.scalar.copy(out_ap, ps[:, :ncol])
                else:  # DRAM destination: stage through SBUF (DMA can't read PSUM)
                    stg = ldp.tile([128, 512], BF16, tag="stgb")
                    nc.scalar.copy(stg[:, :ncol], ps[:, :ncol])
                    nc.sync.dma_start(out=out_ap, in_=stg[:, :ncol])

            # cpT [16, K] = Ws1c.T @ context^T + bs1, then replicate 8x on
            # partitions (emitted first so the lt=0 score phase starts early)
            cpT = cpTrep[0:EH, :]
            for nt4 in range(4):
                nsl = slice(nt4 * 512, (nt4 + 1) * 512)
                ps = psp.tile([128, 512], F32, space="PSUM", tag="mmps")
                nc.tensor.matmul(ps[:EH, :], bs1_sb[:1, :], ones1[:1, :512],
                                 start=True, stop=False)
                for dc in range(4):
                    lh = ldp.tile([128, EH], F32, tag="lhst16")
                    nc.sync.dma_start(out=lh[:],
                                      in_=Ws1c_d[dc * 128:(dc + 1) * 128, :])
                    rh = ldp.tile([128, 512], F32, tag="ctchunk")
                    nc.sync.dma_start(out=rh[:], in_=cT[dc * 128:(dc + 1) * 128, nsl])
                    nc.tensor.matmul(ps[:EH, :], lh[:], rh[:],
                                     start=False, stop=(dc == 3))
                nc.vector.tensor_copy(cpT[:, nsl], ps[:EH, :])
            for ls in range(1, 8):
                nc.sync.dma_start(out=cpTrep[ls * 16:(ls + 1) * 16, :], in_=cpT[:, :])

            # fused u + trio projections (one lhsT load per chunk)
            Wt_t = load_w(Wt_d)
            for lt in range(NT):
                psu = psp.tile([128, 512], F32, space="PSUM", tag="mmps")
                pst3 = ps4p.tile([128, 512], F32, space="PSUM", tag="lamps")
                nc.tensor.matmul(psu[:, :], ones1[0:1, :128], bt_b[:1, :],
                                 start=True, stop=False)
                for dc in range(4):
                    lh = ldp.tile([128, 128], F32, tag="lhst")
                    nc.sync.dma_start(
                        out=lh[:], in_=tT[dc * 128:(dc + 1) * 128,
                                          lt * 128:(lt + 1) * 128])
                    nc.tensor.matmul(psu[:, :], lh[:],
                                     Wt_t[:, dc * D:(dc + 1) * D],
                                     start=False, stop=(dc == 3))
                    nc.tensor.matmul(pst3[:, :48], lh[:],
                                     Wtr3_sb[:, dc * 48:(dc + 1) * 48],
                                     start=(dc == 0), stop=(dc == 3))
                nc.scalar.copy(u_sb[lt][:], psu[:, :])
                nc.scalar.copy(trio[lt][:], pst3[:, :48])
                # tpbT: per-octet score bias columns, partition p = ls*16 + e
                for oc in range(16):
                    nc.sync.dma_start(
                        out=tpbT[:, lt * 16 + oc:lt * 16 + oc + 1],
                        in_=trio[lt][oc * 8:(oc + 1) * 8, 0:EH])

            # ---------- score phase (emitted per lt; lt=0 early) ----------
            def emit_scores(lt):
                scores = medp.tile([128, K], F32, tag="scores", bufs=1)
                for oc in range(16):
                    for hf in range(2):
                        g_sc = medp.tile([128, K // 2], F32, tag="gsc", bufs=1)
                        nc.scalar.activation(
                            g_sc[:], cpTrep[:, hf * 1024:(hf + 1) * 1024], ACTF.Gelu,
                            bias=tpbT[:, lt * 16 + oc:lt * 16 + oc + 1])
                        for nq in range(2):
                            col = hf * 1024 + nq * 512
                            pssc = psp.tile([8, 512], F32, space="PSUM", tag="small")
                            nc.tensor.matmul(pssc[:, :], Ws2bd_sb[:],
                                             g_sc[:, nq * 512:(nq + 1) * 512],
                                             start=True, stop=True)
                            sstg = medp.tile([8, 512], F32, tag="sstg")
                            nc.scalar.copy(sstg[:], pssc[:, :])
                            nc.sync.dma_start(
                                out=scores[oc * 8:(oc + 1) * 8, col:col + 512],
                                in_=sstg[:])
                mx8 = smlp.tile([128, 8], F32, tag="mx8")
                idx = smlp.tile([128, 8], U32, tag="idx", bufs=2)
                nc.vector.max(out=mx8[:], in_=scores[:])
                nc.vector.max_index(out=idx[:], in_max=mx8[:], in_values=scores[:])
                return idx

            idx0 = emit_scores(0)

            # context projection -> Tctx, bf16 single-pass matmuls, fused loads
            Wcb_t = wp.tile([128, 4 * D], BF16, tag="wrhsb")
            for dc in range(4):
                nc.sync.dma_start(out=Wcb_t[:, dc * D:(dc + 1) * D],
                                  in_=Wcb_d[dc * 128:(dc + 1) * 128, :])
            for kt in range(KT):
                psv = psp.tile([128, 512], F32, space="PSUM", tag="mmps")
                psa = ps4p.tile([128, 512], F32, space="PSUM", tag="lamps")
                nc.tensor.matmul(psv[:, :], ones1[32:33, :128], bc_b[:1, :],
                                 start=True, stop=False)
                nc.tensor.matmul(psa[:, :32], ones1[0:1, :128], bacl_sb[:1, :],
                                 start=True, stop=False)
                for dc in range(4):
                    lh = ldp.tile([128, 128], BF16, tag="lhstb")
                    nc.sync.dma_start(
                        out=lh[:], in_=cTb_d[dc * 128:(dc + 1) * 128,
                                            kt * 128:(kt + 1) * 128])
                    nc.tensor.matmul(psv[:, :], lh[:],
                                     Wcb_t[:, dc * D:(dc + 1) * D],
                                     start=False, stop=(dc == 3))
                    nc.tensor.matmul(psa[:, :32], lh[:],
                                     Waclb_sb[:, dc * 32:(dc + 1) * 32],
                                     start=False, stop=(dc == 3))
                stg = ldp.tile([128, CROW], BF16, tag="stgb")
                nc.scalar.copy(stg[:, 0:D], psv[:, :])
                nc.scalar.copy(stg[:, D:D + 32], psa[:, :32])
                nc.sync.dma_start(out=Tctx[kt * 128:(kt + 1) * 128, :],
                                  in_=stg[:, :])


            # ---------- per l-tile ----------
            for lt in range(NT):
                idx = idx0 if lt == 0 else emit_scores(lt)

                # ----- gather context-side rows -----
                gb = gbp.tile([128, WWIN * CROW], BF16, tag="gb")
                gbv = gb[:].rearrange("p (w c) -> p w c", w=8)
                for w in range(WWIN):
                    nc.gpsimd.indirect_dma_start(
                        out=gb[:, w * CROW:(w + 1) * CROW],
                        out_offset=None,
                        in_=Tctx[:, :],
                        in_offset=bass.IndirectOffsetOnAxis(ap=idx[:, w:w + 1], axis=0),
                    )
                # head-major copy of gathered v: vbig[h, w, d] (ACT engine)
                vbig = gbp.tile([128, H * WWIN * HD], BF16, tag="vbig", bufs=1)
                nc.scalar.copy(
                    vbig[:].rearrange("p (h w d) -> p h w d", h=8, w=8),
                    gbv[:, :, 0:D].rearrange("p w (h d) -> p h w d", h=8))

                # ----- per-edge angles -----
                jf = smlp.tile([128, 8], F32, tag="jf")
                nc.vector.tensor_copy(jf[:], idx[:])
                delta = smlp.tile([128, 8], F32, tag="delta")
                nc.vector.tensor_scalar(delta[:], jf[:], lcol_sb[:, lt:lt + 1],
                                        scalar2=None, op0=ALU.subtract)
                ang = medp.tile([128, 8 * 32], F32, tag="ang", bufs=1)
                nc.vector.tensor_tensor(
                    out=ang[:].rearrange("p (w f) -> p w f", w=8),
                    in0=delta[:].unsqueeze(2).to_broadcast((128, 8, 32)),
                    in1=invf_sb[:].unsqueeze(1).to_broadcast((128, 8, 32)),
                    op=ALU.mult)
                # range-reduce to [-pi, pi]: x - 2pi*round(x/2pi), round via
                # the +/- 1.5*2^23 magic-number trick (no mod/floor on DVE ISA)
                MAGIC = 1.5 * 2.0 ** 23
                angt = medp.tile([128, 8 * 32], F32, tag="angt", bufs=1)
                nc.vector.tensor_scalar_mul(angt[:], ang[:], 1.0 / TWO_PI)
                angr = medp.tile([128, 8 * 32], F32, tag="angr", bufs=1)
                nc.vector.tensor_scalar(angr[:], angt[:], MAGIC, scalar2=MAGIC,
                                        op0=ALU.add, op1=ALU.subtract)
                nc.vector.tensor_sub(angt[:], angt[:], angr[:])
                nc.vector.tensor_scalar_mul(ang[:], angt[:], TWO_PI)
                cosb = medp.tile([128, 8 * 32], F32, tag="cosb")
                sinb = medp.tile([128, 8 * 32], F32, tag="sinb")
                nc.scalar.activation(sinb[:], ang[:], ACTF.Sin, scale=-1.0)
                nc.vector.tensor_scalar_mul(angr[:], ang[:], -1.0)
                nc.vector.tensor_max(angt[:], ang[:], angr[:])
                nc.scalar.activation(cosb[:], angt[:], ACTF.Sin, scale=-1.0,
                                     bias=halfpi[:, 0:1])
                # sinb2[w, a, f]: a=0 -> -sin (pairs with -u_hi), a=1 -> +sin
                sinb2 = medp.tile([128, 8 * 2 * 32], F32, tag="sinb2")
                s2v = sinb2[:].rearrange("p (w a f) -> p w a f", w=8, a=2)
                nc.vector.tensor_scalar_mul(
                    s2v[:, :, 0, :], sinb[:].rearrange("p (w f) -> p w f", w=8),
                    -1.0)
                nc.vector.tensor_copy(
                    s2v[:, :, 1, :], sinb[:].rearrange("p (w f) -> p w f", w=8))

                # ----- alphas = softplus(gelu(ta+ca) @ Wa2 + ba2) -----
                ha = smlp.tile([128, 8 * EH], F32, tag="ha")
                nc.vector.tensor_tensor(
                    out=ha[:].rearrange("p (w c) -> p w c", w=8),
                    in0=trio[lt][:, 16:32].unsqueeze(1).to_broadcast((128, 8, EH)),
                    in1=gbv[:, :, D:D + EH],
                    op=ALU.add)
                nc.scalar.activation(ha[:], ha[:], ACTF.Gelu)
                haw = smlp.tile([128, 8 * EH], F32, tag="haw")
                nc.vector.tensor_tensor(
                    out=haw[:].rearrange("p (w c) -> p w c", w=8),
                    in0=ha[:].rearrange("p (w c) -> p w c", w=8),
                    in1=wa2_sb[:].unsqueeze(1).to_broadcast((128, 8, EH)),
                    op=ALU.mult)
                alphas = smlp.tile([128, 8], F32, tag="alphas")
                nc.vector.tensor_reduce(alphas[:], haw[:].rearrange(
                    "p (w c) -> p w c", w=8), axis=AX.X, op=ALU.add)
                softplus(alphas[:], alphas[:], ba2_sb[:, 0:1], smlp, "sptmp")
                # alpharep [128, (w d)] = alphas replicated over d (t/h-invariant)
                alpharep = smlp.tile([128, WWIN * HD], F32, tag="alpharep")
                nc.vector.tensor_copy(
                    alpharep[:].rearrange("p (w d) -> p w d", w=8),
                    alphas[:].unsqueeze(2).to_broadcast((128, 8, HD)))

                # ----- g = gelu(tl + cl) and per-w transposes -----
                gmat = smlp.tile([128, 8 * EH], F32, tag="gmat")
                nc.vector.tensor_tensor(
                    out=gmat[:].rearrange("p (w c) -> p w c", w=8),
                    in0=trio[lt][:, 32:48].unsqueeze(1).to_broadcast((128, 8, EH)),
                    in1=gbv[:, :, D + EH:D + 2 * EH],
                    op=ALU.add)
                nc.scalar.activation(gmat[:], gmat[:], ACTF.Gelu)
                gT4 = gbp.tile([128, 2 * 128], BF16, tag="gT4")  # 2 quads side by side
                nc.vector.memset(gT4[:], 0.0)
                for s4 in range(4):  # bias row (constant 1) for the bl2 fold
                    nc.sync.dma_start(out=gT4[32 * s4 + EH:32 * s4 + EH + 1, :],
                                      in_=onesb[:1, 0:256])
                for w in range(WWIN):
                    q, s = w // 4, w % 4
                    pst = psp.tile([EH, 128], F32, space="PSUM", tag="small")
                    nc.tensor.transpose(
                        out=pst[:, :],
                        in_=gmat[:].rearrange("p (w c) -> p w c", w=8)[:, w, :],
                        identity=ident)
                    nc.vector.tensor_copy(
                        gT4[32 * s:32 * s + EH, q * 128:(q + 1) * 128], pst[:, :])

                # ----- 4-head pipeline groups -----
                for hg in range(H // HG):
                    heads = range(hg * HG, (hg + 1) * HG)
                    lam_h = {}
                    rec_h = {}
                    for h in heads:
                        # Lam layout: (w, r, d) bf16, contiguous
                        Lam = lamp.tile([128, WWIN * R * HD], BF16, tag="lam")
                        lam_h[h] = Lam
                        for w in range(WWIN):
                            q, s = w // 4, w % 4
                            psl = ps4p.tile([128, 512], F32, space="PSUM", tag="lamps")
                            nc.tensor.matmul(
                                psl[:, :], gT4[32 * s:32 * s + 32, q * 128:(q + 1) * 128],
                                Wl2_sb[32 * s:32 * s + 32, h * R * HD:(h + 1) * R * HD],
                                start=True, stop=True, tile_position=(32 * s, 0))
                            nc.scalar.copy(
                                Lam[:, w * R * HD:(w + 1) * R * HD], psl[:, :])
                        # squared row norms, fused across w: sq = Lam^2 (ACT)
                        sq = prodp.tile([128, WWIN * R * HD], BF16, tag="prodb", bufs=3)
                        nc.scalar.activation(sq[:], Lam[:], ACTF.Square)
                        n2 = smlp.tile([128, WWIN * R], F32, tag="n2")
                        nc.vector.tensor_reduce(
                            n2[:].rearrange("p (w r) -> p w r", w=8),
                            sq[:].rearrange("p (w r d) -> p w r d", w=8, r=R),
                            axis=AX.X, op=ALU.add)
                        nrm = smlp.tile([128, WWIN * R], F32, tag="nrm")
                        nc.vector.tensor_scalar_max(nrm[:], n2[:], 1e-24)
                        rec = smlp.tile([128, WWIN * R], F32, tag="rec", bufs=HG)
                        rec_h[h] = rec
                        nc.vector.reciprocal(rec[:], nrm[:])

                    usl = {h: u_sb[lt][:, h * HD:(h + 1) * HD] for h in heads}

                    def emit_diff_e1p(h, t):
                        lam4 = lam_h[h][:].rearrange(
                            "p (w r d) -> p w r d", w=8, r=R)
                        # diff = u_i*cos + rot_half(u_i)*sin - v_j   [128, (w,d)]
                        diff = medp.tile([128, WWIN * HD], BF16, tag="diff",
                                         bufs=HG)
                        d3 = diff[:].rearrange("p (w d) -> p w d", w=8)
                        t0 = medp.tile([128, WWIN * HD], BF16, tag="t0", bufs=3)
                        nc.vector.tensor_tensor(
                            out=t0[:].rearrange("p (w a b) -> p w a b", w=8, a=2),
                            in0=usl[h].rearrange("p (a b) -> p a b", a=2)
                                .unsqueeze(1).to_broadcast((128, 8, 2, 32)),
                            in1=cosb[:].rearrange("p (w f) -> p w f", w=8)
                                .unsqueeze(2).to_broadcast((128, 8, 2, 32)),
                            op=ALU.mult)
                        t1 = medp.tile([128, WWIN * HD], BF16, tag="t0", bufs=3)
                        nc.vector.tensor_tensor(
                            out=t1[:].rearrange("p (w a b) -> p w a b", w=8, a=2),
                            in0=usl[h].rearrange("p (a b) -> p a b", a=2)[:, ::-1, :]
                                .unsqueeze(1).to_broadcast((128, 8, 2, 32)),
                            in1=s2v,
                            op=ALU.mult)
                        nc.vector.tensor_tensor(out=t0[:], in0=t0[:], in1=t1[:],
                                                op=ALU.add)
                        nc.vector.tensor_tensor(
                            out=d3,
                            in0=t0[:].rearrange("p (w d) -> p w d", w=8),
                            in1=vbig[:, h * WWIN * HD:(h + 1) * WWIN * HD]
                                .rearrange("p (w d) -> p w d", w=8),
                            op=ALU.subtract)
                        # einsum1 products on GPSIMD (overlaps DVE)
                        prod = prodp.tile([128, WWIN * R * HD], BF16, tag="prodb",
                                          bufs=3)
                        nc.gpsimd.tensor_tensor(
                            out=prod[:].rearrange("p (w r d) -> p w r d", w=8, r=R),
                            in0=lam4,
                            in1=d3.unsqueeze(2).to_broadcast((128, 8, R, HD)),
                            op=ALU.mult)
                        return diff, prod

                    def emit_e1r_e2p(h, t, prod):
                        lam4 = lam_h[h][:].rearrange(
                            "p (w r d) -> p w r d", w=8, r=R)
                        # einsum1 reduce: Ld[w,r] = sum_d Lam*diff
                        ld = smlp.tile([128, WWIN * R], F32, tag="ld", bufs=3)
                        nc.vector.tensor_reduce(
                            ld[:].rearrange("p (w r) -> p w r", w=8),
                            prod[:].rearrange("p (w r d) -> p w r d", w=8, r=R),
                            axis=AX.X, op=ALU.add)
                        ld2 = smlp.tile([128, WWIN * R], F32, tag="ld2", bufs=3)
                        nc.vector.tensor_tensor(ld2[:], ld[:], rec_h[h][:],
                                                op=ALU.mult)
                        # einsum2 products on GPSIMD: prod2 (w, d, r)
                        prod2 = prodp.tile([128, WWIN * HD * R], BF16, tag="prodb2",
                                           bufs=2)
                        nc.gpsimd.tensor_tensor(
                            out=prod2[:].rearrange("p (w d r) -> p w d r",
                                                   w=8, d=HD),
                            in0=lam4.transpose([0, 1, 3, 2]),
                            in1=ld2[:].rearrange("p (w r) -> p w r", w=8)
                                .unsqueeze(2).to_broadcast((128, 8, HD, R)),
                            op=ALU.mult)
                        return prod2

                    def emit_adiff(h, t, diff):
                        adiff = smlp.tile([128, WWIN * HD], BF16, tag="adiff",
                                          bufs=3)
                        nc.vector.tensor_tensor(adiff[:], diff[:], alpharep[:],
                                                op=ALU.mult)
                        return adiff

                    def emit_tail(h, t, prod2, adiff):
                        stc = slice(t * NT + lt, t * NT + lt + 1)
                        # einsum2 reduce over r, + alpha*diff, then sum over w
                        res = smlp.tile([128, WWIN * HD], F32, tag="res")
                        nc.vector.tensor_reduce(
                            res[:].rearrange("p (w d) -> p w d", w=8),
                            prod2[:].rearrange("p (w d r) -> p w d r",
                                               w=8, d=HD),
                            axis=AX.X, op=ALU.add)
                        nc.vector.tensor_tensor(res[:], res[:], adiff[:],
                                                op=ALU.add)
                        rsum = smlp.tile([128, HD], F32, tag="rsum")
                        nc.vector.tensor_reduce(
                            rsum[:],
                            res[:].rearrange("p (w d) -> p d w", w=8),
                            axis=AX.X, op=ALU.add)
                        # u -= step*rsum
                        nc.vector.scalar_tensor_tensor(
                            out=usl[h], in0=rsum[:], scalar=stpn_sb[:, stc],
                            in1=usl[h], op0=ALU.mult, op1=ALU.add)

                    # software-pipelined 2-iteration t-loop
                    st = {}
                    for h in heads:
                        st[h] = emit_diff_e1p(h, 0)
                    prod2s = {}
                    for h in heads:
                        prod2s[h] = emit_e1r_e2p(h, 0, st[h][1])
                    adiffs = {}
                    for h in heads:
                        adiffs[h] = emit_adiff(h, 0, st[h][0])
                    for h in heads:
                        emit_tail(h, 0, prod2s[h], adiffs[h])
                        st[h] = emit_diff_e1p(h, 1)
                    for h in heads:
                        prod2s[h] = emit_e1r_e2p(h, 1, st[h][1])
                    for h in heads:
                        adiffs[h] = emit_adiff(h, 1, st[h][0])
                    for h in heads:
                        emit_tail(h, 1, prod2s[h], adiffs[h])

            # ---------- output projection: y = u @ Wo + bo ----------
            Wo_t = load_w(Wo_d)
            for lt in range(NT):
                psy = psp.tile([128, 512], F32, space="PSUM", tag="mmps")
                nc.tensor.matmul(psy[:, :], ones1[64:65, :128], bo_b[:1, :],
                                 start=True, stop=False)
                for dc in range(4):
                    pst = psp.tile([128, 128], F32, space="PSUM", tag="small")
                    nc.tensor.transpose(
                        out=pst[:, :], in_=u_sb[lt][:, dc * 128:(dc + 1) * 128],
                        identity=ident)
                    uT = ldp.tile([128, 128], F32, tag="uT")
                    nc.scalar.copy(uT[:], pst[:, :])
                    nc.tensor.matmul(psy[:, :], uT[:], Wo_t[:, dc * D:(dc + 1) * D],
                                     start=False, stop=(dc == 3))
                ystg = ldp.tile([128, 512], F32, tag="stg")
                nc.scalar.copy(ystg[:], psy[:, :])
                nc.sync.dma_start(out=y_d[lt * 128:(lt + 1) * 128, :], in_=ystg[:])

    nc.finalize()
    return nc


def make_in_maps(inputs):
    """Host-side prep: slice/transpose inputs into the 8 per-core input maps."""
    target = np.asarray(inputs["target"], np.float32)
    context = np.asarray(inputs["context"], np.float32)
    Wt = np.asarray(inputs["Wt"], np.float32)
    bt = np.asarray(inputs["bt"], np.float32)
    Wc = np.asarray(inputs["Wc"], np.float32)
    bc = np.asarray(inputs["bc"], np.float32)
    Ws1 = np.asarray(inputs["Ws1"], np.float32)
    bs1 = np.asarray(inputs["bs1"], np.float32)
    Ws2 = np.asarray(inputs["Ws2"], np.float32)
    Wa1 = np.asarray(inputs["Wa1"], np.float32)
    ba1 = np.asarray(inputs["ba1"], np.float32)
    Wa2 = np.asarray(inputs["Wa2"], np.float32)
    ba2 = np.asarray(inputs["ba2"], np.float32)
    Wl1 = np.asarray(inputs["Wl1"], np.float32)
    bl1 = np.asarray(inputs["bl1"], np.float32)
    Wl2 = np.asarray(inputs["Wl2"], np.float32)
    bl2 = np.asarray(inputs["bl2"], np.float32)
    step_sizes = np.asarray(inputs["step_sizes"], np.float32)
    Wo = np.asarray(inputs["Wo"], np.float32)
    bo = np.asarray(inputs["bo"], np.float32)

    import ml_dtypes
    Ws2bd = np.zeros((128, 8), np.float32)  # cast to bf16 below
    for ls in range(8):
        Ws2bd[ls * 16:(ls + 1) * 16, ls] = Ws2[:, 0]
    Wl2s = np.zeros((128, H * R * HD), np.float32)
    for s in range(4):
        Wl2s[32 * s:32 * s + EH, :] = Wl2
        Wl2s[32 * s + EH, :] = bl2
    Wl2s = Wl2s.astype(ml_dtypes.bfloat16)
    invf = (1.0 / (10000.0 ** (np.arange(0, HD, 2, dtype=np.float32) / HD)))[None, :]
    bpack = np.zeros((128, D), np.float32)
    bpack[0] = bt
    bpack[32] = bc
    bpack[64] = bo

    common = dict(
        Wt=Wt, Wcb=Wc.astype(ml_dtypes.bfloat16), Wo=Wo, bpack=bpack,
        Wtr3=np.ascontiguousarray(np.concatenate([Ws1[:D], Wa1[:D], Wl1[:D]], axis=1)),
        Ws1c=np.ascontiguousarray(Ws1[D:]),
        Waclb=np.ascontiguousarray(
            np.concatenate([Wa1[D:], Wl1[D:]], axis=1)).astype(ml_dtypes.bfloat16),
        bs1=bs1[None, :],
        bacl=np.concatenate([ba1, bl1])[None, :],
        Ws2bd=Ws2bd, Wa2=np.ascontiguousarray(Wa2.T),
        ba2=np.asarray(ba2, np.float32).reshape(1, 1), Wl2=Wl2s,
        invf=np.ascontiguousarray(invf, np.float32),
    )

    in_maps = []
    for c in range(8):
        b, rc = c // 4, c % 4
        rows = slice(rc * LC, (rc + 1) * LC)
        stp = np.ascontiguousarray(
            step_sizes[:, rows].reshape(T, NT, 128).transpose(2, 0, 1)
            .reshape(128, T * NT))
        lcol = np.ascontiguousarray(
            (rc * LC + np.arange(LC, dtype=np.float32)).reshape(NT, 128).T)
        m = dict(common)
        cTf = np.ascontiguousarray(context[b].T)
        m.update(
            tT=np.ascontiguousarray(target[b, rows].T),
            cT=cTf, cTb=cTf.astype(ml_dtypes.bfloat16),
            stp=stp, lcol=lcol,
        )
        in_maps.append(m)
    return in_maps


_NC_CACHE = {}


def kernel(**inputs):
    if "nc" not in _NC_CACHE:
        _NC_CACHE["nc"] = build_program()
    nc = _NC_CACHE["nc"]
    in_maps = make_in_maps(inputs)
    res = run_bass_kernel_spmd(nc, in_maps, list(range(8)))
    out = np.empty((B, L, D), np.float32)
    for c in range(8):
        b, rc = c // 4, c % 4
        out[b, rc * LC:(rc + 1) * LC] = res.results[c]["y"]
    return out
